# revision 60
# baseline (speedup 1.0000x reference)
"""Trainium2 Bass kernel for nn_MultiHeadAttention_27711128994021.

Reference math (faithful to the oracle, including its independent-sum einsum):
  q = x@Wq.T+bq ; k = x@Wk.T+bk ; v = x@Wv.T+bv       (B,S,H,D)
  rq, rk = rope(pos, q, k)
  phi_q = elu(rq)+1 ; phi_k = (elu(rk)+1) * notpad
  attn[b,s,h,v] = z[b,h,s] * (sum_q phi_q[b,s,h,q]) * (sum_k kv[b,h,v,k])
    with kv = einsum("bshv,bshk->bhvk", v, phi_k), z = 1/clip(phi_q . k_sum)
  out = attn @ Wo.T + bo

Because q and k are summed independently in the attn einsum, attention is
rank-1 per (b,h):  attn = zq[s,h] * kvsum[h,v]  where
  kvsum[h,v] = sum_s v[s,hv] * psk[s,h],  psk = rowsum(phi_k)
so the V projection collapses:  kvsum = (Wv @ (psk.T @ x).T)_head-diag  and
the out projection collapses to rank-16:  y = zq @ Wo2 + bo with
  Wo2[h,n] = sum_v kvsum[h,v] Wo[n,64h+v].
Only the Q and K projections remain as large matmuls.

Sharding: 8 cores = (batch b, seq half). Per core: 2048 rows of one batch.
Cross-core data: all-reduce (pairs) of xk=psk.T@x [16,1024], k_sum [1024],
psktot [16] — 70KB, hidden behind the Q-path phi production.

All operand preparation (bf16 casts, transposes, per-head [even|odd] row
permutation of Wq/Wk, cos/sin broadcast tables, permuted bias columns) is
done host-side in numpy; the device only streams compute.

Padding mask is applied ADDITIVELY: a rank-1 matmul adds -1e4 to every
masked key column inside the rope PSUM accumulation, so
phi_k = min(exp(v),1)+relu(v) (identical formula to phi_q) comes out zero
for masked rows. elu(x)+1 == min(exp(x),1) + max(x,0) exactly.

phi_q is stashed fully in SBUF so its production has no dependency on the
collective; the qd/z/y tail consumes it j-chunk by j-chunk afterwards,
giving the all-reduce ~35us of slack off the critical path.
"""

import functools

import numpy as np
import ml_dtypes

import concourse.bass as bass
import concourse.mybir as mybir
import concourse.tile as tile
from concourse import bacc
from concourse.bass_utils import run_bass_kernel_spmd

F32 = mybir.dt.float32
BF16 = mybir.dt.bfloat16
FP8 = mybir.dt.float8e4
I32 = mybir.dt.int32
PROJ_FP8 = True   # fp8e4m3 + DoubleRow for the Q/K projections
PDT = FP8 if PROJ_FP8 else BF16
AF = mybir.ActivationFunctionType
ALU = mybir.AluOpType

P = 128
B, S, H, D = 4, 4096, 16, 64
DM = H * D            # 1024
SC = 2048             # seq rows per core
KT = DM // P          # 8 k tiles
FT = DM // P          # 8 feature tiles (2 heads each)
NJ = SC // 512        # 4 s-chunks of 512
NST = SC // P         # 16 seq tiles of 128
EPS = 1e-6
N_CORES = 8
CC_XK, CC_KS, CC_PT = P * KT * H, P * FT, 16   # collective bundle sections
CC_LEN = CC_XK + CC_KS + CC_PT

bf = ml_dtypes.bfloat16
f8 = ml_dtypes.float8_e4m3fn
pdt = f8 if PROJ_FP8 else bf


def build_program(collective=True):
    nc = bacc.Bacc(
        "TRN2", target_bir_lowering=False, debug=False, num_devices=N_CORES
    )

    # ---- I/O (everything already laid out / cast host-side) ----
    xT_in = nc.dram_tensor("xT", [P, 2, KT, 1024], PDT, kind="ExternalInput").ap()
    xn_in = nc.dram_tensor("xn", [P, NST, DM], BF16, kind="ExternalInput").ap()
    wqT_in = nc.dram_tensor("wqT", [P, FT, KT, P], PDT, kind="ExternalInput").ap()
    wkT_in = nc.dram_tensor("wkT", [P, FT, KT, P], PDT, kind="ExternalInput").ap()
    wvT_in = nc.dram_tensor("wvT", [P, KT, DM], BF16, kind="ExternalInput").ap()
    woT_in = nc.dram_tensor("woT", [P, KT, DM], BF16, kind="ExternalInput").ap()
    # trig[:,0,:]=cos, [:,1,:]=sin ; cpack = [ident|psign|ones3] ;
    # mnegc = [-1e4*mask | ones-row] ; bkq = [bkT|bqT]
    trig_in = nc.dram_tensor("trig", [P, 2, SC], BF16, kind="ExternalInput").ap()
    cpack_in = nc.dram_tensor("cpack", [P, 259], BF16, kind="ExternalInput").ap()
    mnegc_in = nc.dram_tensor("mnegc", [1, SC + P], BF16, kind="ExternalInput").ap()
    bkq_in = nc.dram_tensor("bkq", [P, 2 * FT], F32, kind="ExternalInput").ap()
    bvb_in = nc.dram_tensor("bvb", [1, DM], BF16, kind="ExternalInput").ap()
    bob_in = nc.dram_tensor("bob", [1, DM], BF16, kind="ExternalInput").ap()
    y_out = nc.dram_tensor("y", [SC, DM], F32, kind="ExternalOutput").ap()

    with tile.TileContext(nc) as tc:
        with (
            tc.tile_pool(name="const", bufs=1) as cp,
            tc.tile_pool(name="work", bufs=3 if PROJ_FP8 else 2) as wp,
            tc.tile_pool(name="ypool", bufs=3) as yp,
            tc.tile_pool(name="pbig", bufs=6, space="PSUM") as pb,
            tc.tile_pool(name="psmall", bufs=2, space="PSUM") as pp_s,
            tc.tile_pool(name="dram", bufs=1, space="DRAM") as dp,
        ):
            cc_i = dp.tile([CC_LEN], F32, tag="cc_i")
            cc_o = dp.tile([CC_LEN], F32, tag="cc_o")

            # ---- SBUF residents ----
            xT = cp.tile([P, 2, KT, 1024], PDT, tag="xT")
            xn = cp.tile([P, NST, DM], BF16, tag="xn")
            wqT = cp.tile([P, FT, KT, P], PDT, tag="wqT")
            wkT = cp.tile([P, FT, KT, P], PDT, tag="wkT")
            wvT = cp.tile([P, KT, DM], BF16, tag="wvT")
            woT = cp.tile([P, KT, DM], BF16, tag="woT")
            trig = cp.tile([P, 2, SC], BF16, tag="trig")
            cpack = cp.tile([P, 259], BF16, tag="cpack")
            mnegc = cp.tile([1, SC + P], BF16, tag="mnegc")
            bkq = cp.tile([P, 2 * FT], F32, tag="bkq")
            bkT = bkq[:, 0:FT]
            bqT = bkq[:, FT:2 * FT]
            bvb = cp.tile([1, DM], BF16, tag="bvb")
            phiq_all = cp.tile([P, FT, SC], BF16, tag="phiq_all")

            # ---- wave 1 loads: big K-path operands on the SP/HWDGE queue,
            # small tables on the Pool/SWDGE queue (parallel issue paths) ----
            nc.sync.dma_start(wkT[:, 0:1], wkT_in[:, 0:1])
            nc.sync.dma_start(xT[:, 0, 0:4], xT_in[:, 0, 0:4])
            nc.sync.dma_start(xT[:, 0, 4:KT], xT_in[:, 0, 4:KT])
            nc.sync.dma_start(wkT[:, 1:4], wkT_in[:, 1:4])
            nc.sync.dma_start(wkT[:, 4:FT], wkT_in[:, 4:FT])
            nc.sync.dma_start(xT[:, 1, 0:4], xT_in[:, 1, 0:4])
            nc.sync.dma_start(xT[:, 1, 4:KT], xT_in[:, 1, 4:KT])
            nc.gpsimd.dma_start(trig[:, :, 0:1024], trig_in[:, :, 0:1024])
            nc.gpsimd.dma_start(bkq[:], bkq_in)
            nc.gpsimd.dma_start(cpack[:], cpack_in)
            nc.gpsimd.dma_start(mnegc[:], mnegc_in)
            nc.gpsimd.dma_start(trig[:, :, 1024:SC], trig_in[:, :, 1024:SC])
            # preload the Exp activation table while ACT is otherwise idle
            actwarm = cp.tile([1, 2], BF16, tag="actwarm")
            nc.scalar.activation(actwarm[:], trig[0:1, 0, 0:2], AF.Exp)

            # accumulators
            ksum_parts = cp.tile([P, FT * NJ], F32, tag="ksum_parts")
            psk_nat = cp.tile([P, NST, H], BF16, tag="psk_nat")
            zqext = cp.tile([H + 1, SC], BF16, tag="zqext")
            nc.vector.memset(zqext[:], 1.0)
            wo2ext = cp.tile([H + 1, DM], BF16, tag="wo2ext")
            nc.gpsimd.dma_start(wo2ext[H:H + 1, :], bob_in)

            # ---- wave 2 loads (Pool/SWDGE queue), emitted mid-K-path.
            # The scheduler reorders independent queue entries, so gate each
            # transfer behind an early K-path product with a real WAW dep
            # (corner-copy into the dst).
            def wave2():
                def gated(corner, dst, src):
                    nc.gpsimd.tensor_copy(corner, psk_nat[0:2, 4, 0:2])
                    nc.gpsimd.dma_start(dst, src)

                gated(wqT[0:2, 0, 0, 0:2], wqT[:], wqT_in)
                for sg in range(4):
                    gated(
                        xn[0:2, 4 * sg, 0:2],
                        xn[:, 4 * sg:4 * sg + 4], xn_in[:, 4 * sg:4 * sg + 4]
                    )
                gated(wvT[0:2, 0, 0:2], wvT[:], wvT_in)
                gated(woT[0:2, 0, 0:2], woT[:], woT_in)
                nc.gpsimd.dma_start(bvb[:], bvb_in)

            def unit_front(path, t, j, s12_pool=False):
                """proj chain + bias evac + cos/sin products, 512-wide."""
                jsl = slice(j * 512, (j + 1) * 512)
                J, jh = j // 2, (j % 2) * 512
                wT = wkT if path == "k" else wqT
                bofs = 0 if path == "k" else FT
                projP = pb.tile([P, 512], F32, tag="pb")
                if PROJ_FP8:
                    for kp in range(KT // 2):
                        nc.tensor.matmul(
                            projP[:], wT[:, t, 2 * kp:2 * kp + 2],
                            xT[:, J, 2 * kp:2 * kp + 2, jh:jh + 512],
                            start=(kp == 0), stop=(kp == KT // 2 - 1),
                            perf_mode=mybir.MatmulPerfMode.DoubleRow,
                        )
                else:
                    for kt in range(KT):
                        nc.tensor.matmul(
                            projP[:], wT[:, t, kt], xT[:, J, kt, jh:jh + 512],
                            start=(kt == 0), stop=(kt == KT - 1),
                        )
                ck = wp.tile([P, 1, 512], BF16, tag="ck")
                nc.scalar.activation(
                    ck[:, 0], projP[:], AF.Identity,
                    bias=bkq[:, bofs + t:bofs + t + 1],
                )
                # s12[:,0,:] = ck*cos, s12[:,1,:] = ck*sin in one op via a
                # stride-0 broadcast of ck over the middle dim
                s12 = wp.tile([P, 2, 512], BF16, tag="s12")
                eng = nc.gpsimd if s12_pool else nc.vector
                eng.tensor_tensor(
                    s12[:], ck[:].broadcast_to([P, 2, 512]), trig[:, :, jsl],
                    ALU.mult,
                )
                return (path, t, j, projP, s12)

            def unit_back(state, out_phi, stt_pool=False, relu_act=True):
                """rope (reusing the proj PSUM slot) + feature map."""
                path, t, j, projP, s12 = state
                nc.tensor.matmul(
                    projP[:], cpack[:, 0:P], s12[:, 0], start=True, stop=False
                )
                nc.tensor.matmul(
                    projP[:], cpack[:, P:2 * P], s12[:, 1],
                    start=False, stop=(path == "q"),
                )
                if path == "k":
                    nc.tensor.matmul(
                        projP[:], mnegc[0:1, SC:SC + P],
                        mnegc[0:1, j * 512:(j + 1) * 512],
                        start=False, stop=True,
                    )
                e = wp.tile([P, 512], BF16, tag="e")
                nc.scalar.activation(e[:], projP[:], AF.Exp)
                r = wp.tile([P, 512], BF16, tag="r")
                if relu_act:
                    nc.scalar.activation(r[:], projP[:], AF.Relu)
                else:
                    nc.vector.tensor_scalar_max(r[:], projP[:], 0.0)
                acc = (
                    ksum_parts[:, t * NJ + j:t * NJ + j + 1]
                    if path == "k" else None
                )
                eng = nc.gpsimd if stt_pool else nc.vector
                eng.scalar_tensor_tensor(
                    out_phi, e[:], 1.0, r[:], ALU.min, ALU.add, accum_out=acc
                )

            # ============ K path: software-pipelined, j-outer ============
            def psk_stage(phik, t, j):
                pskT = pp_s.tile([P, 8], F32, tag="small")
                for sub in range(4):
                    nc.tensor.matmul(
                        pskT[:, 2 * sub:2 * sub + 2],
                        phik[:, sub * P:(sub + 1) * P],
                        cpack[:, 2 * P:2 * P + 2],
                    )
                nc.vector.tensor_copy(
                    psk_nat[:, 4 * j:4 * j + 4, 2 * t:2 * t + 2],
                    pskT.rearrange("p (st hh) -> p st hh", hh=2),
                )

            def k_back(state):
                path, t, j, _, _ = state
                phik = wp.tile([P, 512], BF16, tag="phik")
                unit_back(state, phik[:], relu_act=(t % 2 == 0))
                psk_stage(phik, t, j)

            import collections as _c
            kq = _c.deque()
            for j in range(NJ):
                for t in range(FT):
                    kq.append(unit_front("k", t, j, s12_pool=(t % 4 == 3 and j > 0)))
                    if len(kq) > 2:
                        st = kq.popleft()
                        k_back(st)
                        if st[1:3] == (0, 1):
                            wave2()
            qpend = unit_front("q", 0, 0, s12_pool=False)
            while kq:
                k_back(kq.popleft())

            # ksum_flat = sum of the 4 j-chunks
            kv4 = ksum_parts.rearrange("p (t j) -> p t j", j=NJ)
            kst1 = cp.tile([P, FT], F32, tag="kst1")
            kst2 = cp.tile([P, FT], F32, tag="kst2")
            ksum_flat = cp.tile([P, FT], F32, tag="ksum_flat")
            nc.vector.tensor_tensor(kst1[:], kv4[:, :, 0], kv4[:, :, 1], ALU.add)
            nc.vector.tensor_tensor(kst2[:], kv4[:, :, 2], kv4[:, :, 3], ALU.add)
            nc.vector.tensor_tensor(ksum_flat[:], kst1[:], kst2[:], ALU.add)

            # xk in transposed form xkT[p, kt, h] = sum_s x[s, kt*128+p] psk[s, h]
            xkP = pp_s.tile([P, KT * H], F32, tag="small")
            xkv = xkP.rearrange("p (kt h) -> p kt h", h=H)
            for kt in range(KT):
                for st in range(NST):
                    nc.tensor.matmul(
                        xkv[:, kt], xn[:, st, kt * P:(kt + 1) * P],
                        psk_nat[:, st, :],
                        start=(st == 0), stop=(st == NST - 1),
                    )
            xk_sb = cp.tile([P, KT * H], F32, tag="xk_sb")
            nc.vector.tensor_copy(xk_sb[:], xkP[:])
            # psktot = colsum of psk
            ptP = pp_s.tile([1, H], F32, tag="small")
            for st in range(NST):
                nc.tensor.matmul(
                    ptP[:], cpack[:, 2 * P + 2:2 * P + 3], psk_nat[:, st, :],
                    start=(st == 0), stop=(st == NST - 1),
                )
            psktot_f = cp.tile([1, H], F32, tag="psktot_f")
            nc.vector.tensor_copy(psktot_f[:], ptP[:])

            # ============ collective (pairs share a batch) ============
            with nc.allow_non_contiguous_dma(reason="70KB collective bundle"):
                nc.sync.dma_start(
                    cc_i[0:CC_XK].rearrange("(a b) -> a b", a=P), xk_sb[:]
                )
                nc.sync.dma_start(
                    cc_i[CC_XK:CC_XK + CC_KS].rearrange("(a b) -> a b", a=P),
                    ksum_flat[:],
                )
                nc.sync.dma_start(
                    cc_i[CC_XK + CC_KS:CC_LEN].rearrange("(a b) -> a b", a=1),
                    psktot_f[:],
                )
            if collective:
                nc.gpsimd.collective_compute(
                    "AllReduce",
                    ALU.add,
                    replica_groups=[[0, 1], [2, 3], [4, 5], [6, 7]],
                    ins=[cc_i.opt()],
                    outs=[cc_o.opt()],
                )
            else:  # timing-model variant: TimelineSim can't model collectives
                nc.sync.dma_start(cc_o[:], cc_i[:])

            # ============ Q path: phi production (collective-independent) ==
            def q_front(t, j):
                return unit_front("q", t, j, s12_pool=(t % 4 == 1))

            def q_back(state):
                _, t, j, _, _ = state
                unit_back(
                    state, phiq_all[:, t, j * 512:(j + 1) * 512],
                    relu_act=(t % 4 == 2),
                )

            qq = _c.deque([qpend])
            for j in range(0, 2):
                for t in range(FT):
                    if j == 0 and t == 0:
                        continue
                    qq.append(q_front(t, j))
                    if len(qq) > 2:
                        q_back(qq.popleft())

            # ---- collective consumers: emitted ~2 j-chunks (35us) after the
            # all-reduce was issued, so even a slow collective is off the
            # critical path by the time these hit the engine queues ----
            xk_r = cp.tile([P, KT * H], F32, tag="xk_r")
            ksum_r = cp.tile([P, FT], F32, tag="ksum_r")
            psktot_r = cp.tile([1, H], F32, tag="psktot_r")
            with nc.allow_non_contiguous_dma(reason="70KB collective bundle"):
                nc.sync.dma_start(
                    xk_r[:], cc_o[0:CC_XK].rearrange("(a b) -> a b", a=P)
                )
                nc.sync.dma_start(
                    ksum_r[:],
                    cc_o[CC_XK:CC_XK + CC_KS].rearrange("(a b) -> a b", a=P),
                )
                nc.sync.dma_start(
                    psktot_r[:],
                    cc_o[CC_XK + CC_KS:CC_LEN].rearrange("(a b) -> a b", a=1),
                )
            xkT8 = cp.tile([P, KT, H], BF16, tag="xkT8")
            nc.vector.tensor_copy(
                xkT8[:], xk_r.rearrange("p (kt h) -> p kt h", h=H)
            )
            psktot_rb = cp.tile([1, H], BF16, tag="psktot_rb")
            nc.vector.tensor_copy(psktot_rb[:], psktot_r[:])
            # qones[p, m, t]: m 0/1 = head-half ones, m 2/3 = ksum halves
            qones = cp.tile([P, 4, FT], BF16, tag="qones")
            nc.vector.memset(qones[:], 0.0)
            nc.vector.memset(qones[0:64, 0, :], 1.0)
            nc.vector.memset(qones[64:P, 1, :], 1.0)
            nc.vector.tensor_copy(qones[0:64, 2, :], ksum_r[0:64, :])
            nc.vector.tensor_copy(qones[64:P, 3, :], ksum_r[64:P, :])

            # kvsum / Wo2
            kvsum_f = cp.tile([P, FT], F32, tag="kvsum_f")
            for t in range(FT):
                tsl = slice(t * P, (t + 1) * P)
                kvP = pp_s.tile([P, H], F32, tag="small")
                for kt in range(KT):
                    nc.tensor.matmul(
                        kvP[:], wvT[:, kt, tsl], xkT8[:, kt, :],
                        start=(kt == 0), stop=False,
                    )
                nc.tensor.matmul(
                    kvP[:], bvb[:, tsl], psktot_rb[:],
                    start=False, stop=True,
                )
                nc.vector.tensor_copy(
                    kvsum_f[0:64, t:t + 1], kvP[0:64, 2 * t:2 * t + 1]
                )
                nc.vector.tensor_copy(
                    kvsum_f[64:P, t:t + 1], kvP[64:P, 2 * t + 1:2 * t + 2]
                )
            kvsel = cp.tile([P, KT, H], BF16, tag="kvsel")
            nc.vector.memset(kvsel[:], 0.0)
            kvselv = kvsel.rearrange("p kt h -> p (kt h)")
            # element (t, 2t [+1]) of the [8,16] grid = flat index 18t [+1]
            nc.vector.tensor_copy(
                kvselv[0:64, 0:127:18], kvsum_f[0:64, :]
            )
            nc.vector.tensor_copy(
                kvselv[64:P, 1:128:18], kvsum_f[64:P, :]
            )
            for half in range(2):
                hsl = slice(half * 512, (half + 1) * 512)
                w2P = pp_s.tile([H, 512], F32, tag="small")
                for kt in range(KT):
                    nc.tensor.matmul(
                        w2P[:], kvsel[:, kt, :], woT[:, kt, hsl],
                        start=(kt == 0), stop=(kt == KT - 1),
                    )
                nc.scalar.copy(wo2ext[0:H, hsl], w2P[:])

            # ============ qd/z/y tail for one j-chunk ====================
            qd_nat = cp.tile([P, NST * FT * 4], F32, tag="qd_nat")
            qdv = qd_nat.rearrange("p (st t m) -> p st t m", st=NST, t=FT)
            den_cl = cp.tile([P, 256], F32, tag="den_cl")
            zr = cp.tile([P, 256], F32, tag="zr")
            zq_c = cp.tile([P, 256], BF16, tag="zq_c")
            zqv = zq_c.rearrange("p (st t hh) -> p st t hh", st=NST, t=FT)
            zrv = zr.rearrange("p (st t hh) -> p st t hh", st=NST, t=FT)
            dclv = den_cl.rearrange("p (st t hh) -> p st t hh", st=NST, t=FT)

            def tail_qd(j):
                qdP = pp_s.tile([P, FT * 16], F32, tag="small")
                qdPv = qdP.rearrange("p (t s m) -> p t s m", t=FT, s=4)
                for t in range(FT):
                    for sub in range(4):
                        st = 4 * j + sub
                        nc.tensor.matmul(
                            qdPv[:, t, sub, :],
                            phiq_all[:, t, st * P:(st + 1) * P],
                            qones[:, :, t],
                        )
                sts = slice(4 * j, 4 * (j + 1))
                nc.vector.tensor_copy(
                    qdv[:, sts, :, :],
                    qdP.rearrange("p (t s m) -> p s t m", t=FT, s=4),
                )

            def tail_z(j):
                sts = slice(4 * j, 4 * (j + 1))
                zsl = slice(64 * j, 64 * (j + 1))
                sts = slice(4 * j, 4 * (j + 1))
                nc.vector.tensor_scalar_max(
                    dclv[:, sts], qdv[:, sts, :, 2:4], EPS
                )
                nc.vector.reciprocal(zr[:, zsl], den_cl[:, zsl])
                nc.vector.tensor_tensor(
                    zqv[:, sts], zrv[:, sts], qdv[:, sts, :, 0:2], ALU.mult
                )
                for sub in range(4):
                    st = 4 * j + sub
                    ssl = slice(st * P, (st + 1) * P)
                    zP = pp_s.tile([H, P], BF16, tag="small")
                    nc.tensor.transpose(
                        zP[:], zq_c[:, st * H:(st + 1) * H], cpack[:, 0:P]
                    )
                    nc.scalar.copy(zqext[0:H, ssl], zP[:])

            def tail_y(j):
                for sub in range(4):
                    st = 4 * j + sub
                    ssl = slice(st * P, (st + 1) * P)
                    ysb = yp.tile([P, DM], F32, tag="ysb")
                    for half in range(2):
                        hsl = slice(half * 512, (half + 1) * 512)
                        yP = pp_s.tile([P, 512], F32, tag="small")
                        nc.tensor.matmul(yP[:], zqext[:, ssl], wo2ext[:, hsl])
                        if half == 0:
                            nc.vector.tensor_copy(ysb[:, hsl], yP[:])
                        else:
                            nc.scalar.copy(ysb[:, hsl], yP[:])
                    nc.sync.dma_start(y_out[ssl, :], ysb[:])

            def tail(j):
                tail_qd(j)
                tail_z(j)
                tail_y(j)

            tail(0)
            for t in range(FT):
                qq.append(q_front(t, 2))
                q_back(qq.popleft())
            tail(1)
            for t in range(0, 4):
                qq.append(q_front(t, 3))
                q_back(qq.popleft())
            tail_qd(2)
            for t in range(4, FT):
                qq.append(q_front(t, 3))
                q_back(qq.popleft())
            while qq:
                q_back(qq.popleft())
            tail_z(2)
            tail_qd(3)
            tail_y(2)
            tail_z(3)
            tail_y(3)

    nc._dbg = dict(
        psk_nat=psk_nat, ksum_parts=ksum_parts, ksum_flat=ksum_flat,
        xk_sb=xk_sb, psktot_f=psktot_f, xkT8=xkT8, ksum_r=ksum_r,
        kvsum_f=kvsum_f, kvsel=kvsel, wo2ext=wo2ext, qones=qones,
        qd_nat=qd_nat, zq_c=zq_c, phiq_all=phiq_all, zqext=zqext,
    )
    nc.finalize()
    return nc


def _consts():
    psign = np.zeros((P, P), np.float32)
    for h in range(2):
        for i in range(32):
            psign[h * 64 + 32 + i, h * 64 + i] = -1.0   # even' = .. - s*odd
            psign[h * 64 + i, h * 64 + 32 + i] = 1.0    # odd'  = .. + s*even
    ones3 = np.zeros((P, 3), np.float32)
    ones3[0:64, 0] = 1.0
    ones3[64:P, 1] = 1.0
    ones3[:, 2] = 1.0
    cpack = np.concatenate(
        [np.eye(P, dtype=np.float32), psign, ones3], axis=1
    ).astype(bf)
    return {"cpack": np.ascontiguousarray(cpack)}


def _perm_rows(w):
    """Per-head [even|odd] row permutation: dest row h*64+pr*32+i comes from
    source row h*64+2i+pr."""
    return np.ascontiguousarray(
        w.reshape(H, 32, 2, -1).transpose(0, 2, 1, 3).reshape(DM, -1)
    )


@functools.lru_cache(maxsize=1)
def _program():
    return build_program()


def make_in_maps(inputs):
    consts = _consts()
    f32 = np.float32

    def wT_tmajor(w):  # [128, t, kt, 128] from permuted W
        a = np.ascontiguousarray(_perm_rows(np.asarray(w, f32)).T.astype(pdt))
        return np.ascontiguousarray(
            a.reshape(KT, P, FT, P).transpose(1, 2, 0, 3)
        )

    def wT_plain(w):   # [128, kt, 1024] = W.T tiles
        a = np.ascontiguousarray(np.asarray(w, f32).T.astype(bf))
        return np.ascontiguousarray(a.reshape(KT, P, DM).transpose(1, 0, 2))

    def bT_perm(b):    # [128, t] f32 permuted bias columns
        bp = np.asarray(b, f32).reshape(H, 32, 2).transpose(0, 2, 1).reshape(DM)
        return np.ascontiguousarray(bp.reshape(FT, P).T)

    bkq = np.concatenate(
        [bT_perm(inputs["bk"]), bT_perm(inputs["bq"])], axis=1
    )
    shared = {
        "wqT": wT_tmajor(inputs["Wq"]),
        "wkT": wT_tmajor(inputs["Wk"]),
        "wvT": wT_plain(inputs["Wv"]),
        "woT": wT_plain(inputs["Wo"]),
        "bkq": np.ascontiguousarray(bkq),
        "bvb": np.asarray(inputs["bv"], f32).astype(bf).reshape(1, DM),
        "bob": np.asarray(inputs["bo"], f32).astype(bf).reshape(1, DM),
        **consts,
    }
    x = np.asarray(inputs["x"], f32)
    pos = np.asarray(inputs["rotary_pos_enc"], f32)
    mask = np.asarray(inputs["padding_mask"], np.int32)
    # cos/sin broadcast tables per seq half: trig[:,0,:]=cos, [:,1,:]=sin
    trig = []
    for hf in range(2):
        sl = slice(hf * SC, (hf + 1) * SC)
        tt = np.zeros((P, 2, SC), bf)
        tt[:, 0, :] = np.tile(pos[sl, 0, 32:64].T.astype(bf), (4, 1))
        tt[:, 1, :] = np.tile(pos[sl, 0, 0:32].T.astype(bf), (4, 1))
        trig.append(np.ascontiguousarray(tt))
    in_maps = []
    for c in range(N_CORES):
        b_, hf = c // 2, c % 2
        sl = slice(hf * SC, (hf + 1) * SC)
        xc = np.ascontiguousarray(x[b_, sl])                     # [2048,1024]
        xTc = np.ascontiguousarray(
            xc.T.astype(pdt).reshape(KT, P, 2, 1024).transpose(1, 2, 0, 3)
        )
        xnc = np.ascontiguousarray(
            xc.astype(bf).reshape(NST, P, DM).transpose(1, 0, 2)
        )
        mnegc = np.ones((1, SC + P), np.float32)
        mnegc[0, 0:SC] = mask[b_, sl].astype(f32) * -1e4
        in_maps.append(
            {
                "xT": xTc,
                "xn": xnc,
                "trig": trig[hf],
                "mnegc": np.ascontiguousarray(mnegc.astype(bf)),
                **shared,
            }
        )
    return in_maps


def run(inputs, **kwargs):
    nc = _program()
    in_maps = make_in_maps(inputs)
    res = run_bass_kernel_spmd(
        nc, in_maps, core_ids=list(range(N_CORES)), **kwargs
    )
    out = np.zeros((B, S, DM), np.float32)
    for c in range(N_CORES):
        b_, hf = c // 2, c % 2
        out[b_, hf * SC:(hf + 1) * SC, :] = res.results[c]["y"]
    return out, res


def kernel(**inputs) -> np.ndarray:
    out, _ = run(inputs)
    return out


# revision 70
# speedup vs baseline: 1.0254x; 1.0254x over previous
"""Trainium2 Bass kernel for nn_MultiHeadAttention_27711128994021.

Reference math (faithful to the oracle, including its independent-sum einsum):
  q = x@Wq.T+bq ; k = x@Wk.T+bk ; v = x@Wv.T+bv       (B,S,H,D)
  rq, rk = rope(pos, q, k)
  phi_q = elu(rq)+1 ; phi_k = (elu(rk)+1) * notpad
  attn[b,s,h,v] = z[b,h,s] * (sum_q phi_q[b,s,h,q]) * (sum_k kv[b,h,v,k])
    with kv = einsum("bshv,bshk->bhvk", v, phi_k), z = 1/clip(phi_q . k_sum)
  out = attn @ Wo.T + bo

Because q and k are summed independently in the attn einsum, attention is
rank-1 per (b,h):  attn = zq[s,h] * kvsum[h,v]  where
  kvsum[h,v] = sum_s v[s,hv] * psk[s,h],  psk = rowsum(phi_k)
so the V projection collapses:  kvsum = (Wv @ (psk.T @ x).T)_head-diag  and
the out projection collapses to rank-16:  y = zq @ Wo2 + bo with
  Wo2[h,n] = sum_v kvsum[h,v] Wo[n,64h+v].
Only the Q and K projections remain as large matmuls.

Sharding: 8 cores = (batch b, seq half). Per core: 2048 rows of one batch.
Cross-core data: all-reduce (pairs) of xk=psk.T@x [16,1024], k_sum [1024],
psktot [16] — 70KB, hidden behind the Q-path phi production.

All operand preparation (bf16 casts, transposes, per-head [even|odd] row
permutation of Wq/Wk, cos/sin broadcast tables, permuted bias columns) is
done host-side in numpy; the device only streams compute.

Padding mask is applied ADDITIVELY: a rank-1 matmul adds -1e4 to every
masked key column inside the rope PSUM accumulation, so
phi_k = min(exp(v),1)+relu(v) (identical formula to phi_q) comes out zero
for masked rows. elu(x)+1 == min(exp(x),1) + max(x,0) exactly.

phi_q is stashed fully in SBUF so its production has no dependency on the
collective; the qd/z/y tail consumes it j-chunk by j-chunk afterwards,
giving the all-reduce ~35us of slack off the critical path.

Schedule: Q/K projections run as fp8e4m3 DoubleRow matmuls (4 chained
k-pair passes); each (t, j) unit is software-pipelined two deep
(proj+evac+s12 fronts run ahead of rope+exp+relu+phi backs), with rope
accumulating into the *same* PSUM slot the projection used, so one
1-bank PSUM slot per unit gives six units in flight. Work is spread
across engines: ACT does evac/exp (+some relu), DVE does s12/relu/phi
(min,add with ksum accum), the Pool engine takes a quarter of the
s12 products (it cannot run TensorScalarPtr or touch PSUM on real HW).
Weight/x loads stream in two waves (SP/HWDGE for big K operands,
Pool/SWDGE for tables), with wave 2 gated behind an early K product to
keep the shared DMA engines clear during ramp.
"""

import functools

import numpy as np
import ml_dtypes

import concourse.bass as bass
import concourse.mybir as mybir
import concourse.tile as tile
from concourse import bacc
from concourse.bass_utils import run_bass_kernel_spmd

F32 = mybir.dt.float32
BF16 = mybir.dt.bfloat16
FP8 = mybir.dt.float8e4
I32 = mybir.dt.int32
PROJ_FP8 = True   # fp8e4m3 + DoubleRow for the Q/K projections
PDT = FP8 if PROJ_FP8 else BF16
AF = mybir.ActivationFunctionType
ALU = mybir.AluOpType

P = 128
B, S, H, D = 4, 4096, 16, 64
DM = H * D            # 1024
SC = 2048             # seq rows per core
KT = DM // P          # 8 k tiles
FT = DM // P          # 8 feature tiles (2 heads each)
NJ = SC // 512        # 4 s-chunks of 512
NST = SC // P         # 16 seq tiles of 128
EPS = 1e-6
N_CORES = 8
CC_XK, CC_KS, CC_PT = P * KT * H, P * FT, 16   # collective bundle sections
CC_LEN = CC_XK + CC_KS + CC_PT

bf = ml_dtypes.bfloat16
f8 = ml_dtypes.float8_e4m3fn
pdt = f8 if PROJ_FP8 else bf


def build_program(collective=True):
    nc = bacc.Bacc(
        "TRN2", target_bir_lowering=False, debug=False, num_devices=N_CORES
    )

    # ---- I/O (everything already laid out / cast host-side) ----
    xT_in = nc.dram_tensor("xT", [P, 2, KT, 1024], PDT, kind="ExternalInput").ap()
    xn_in = nc.dram_tensor("xn", [P, NST, DM], BF16, kind="ExternalInput").ap()
    wqT_in = nc.dram_tensor("wqT", [P, FT, KT, P], PDT, kind="ExternalInput").ap()
    wkT_in = nc.dram_tensor("wkT", [P, FT, KT, P], PDT, kind="ExternalInput").ap()
    wvT_in = nc.dram_tensor("wvT", [P, KT, DM], BF16, kind="ExternalInput").ap()
    woT_in = nc.dram_tensor("woT", [P, KT, DM], BF16, kind="ExternalInput").ap()
    # trig[:,0,:]=cos, [:,1,:]=sin ; cpack = [ident|psign|ones3] ;
    # mnegc = [-1e4*mask | ones-row] ; bkq = [bkT|bqT]
    trig_in = nc.dram_tensor("trig", [P, 2, SC], BF16, kind="ExternalInput").ap()
    cpack_in = nc.dram_tensor("cpack", [P, 259], BF16, kind="ExternalInput").ap()
    mnegc_in = nc.dram_tensor("mnegc", [1, SC + P], BF16, kind="ExternalInput").ap()
    bkq_in = nc.dram_tensor("bkq", [P, 2 * FT], F32, kind="ExternalInput").ap()
    bvb_in = nc.dram_tensor("bvb", [1, DM], BF16, kind="ExternalInput").ap()
    bob_in = nc.dram_tensor("bob", [1, DM], BF16, kind="ExternalInput").ap()
    y_out = nc.dram_tensor("y", [SC, DM], F32, kind="ExternalOutput").ap()

    with tile.TileContext(nc) as tc:
        with (
            tc.tile_pool(name="const", bufs=1) as cp,
            tc.tile_pool(name="work", bufs=3 if PROJ_FP8 else 2) as wp,
            tc.tile_pool(name="ypool", bufs=3) as yp,
            tc.tile_pool(name="pbig", bufs=6, space="PSUM") as pb,
            tc.tile_pool(name="psmall", bufs=2, space="PSUM") as pp_s,
            tc.tile_pool(name="dram", bufs=1, space="DRAM") as dp,
        ):
            cc_i = dp.tile([CC_LEN], F32, tag="cc_i")
            cc_o = dp.tile([CC_LEN], F32, tag="cc_o")

            # ---- SBUF residents ----
            xT = cp.tile([P, 2, KT, 1024], PDT, tag="xT")
            xn = cp.tile([P, NST, DM], BF16, tag="xn")
            wqT = cp.tile([P, FT, KT, P], PDT, tag="wqT")
            wkT = cp.tile([P, FT, KT, P], PDT, tag="wkT")
            wvT = cp.tile([P, KT, DM], BF16, tag="wvT")
            woT = cp.tile([P, KT, DM], BF16, tag="woT")
            trig = cp.tile([P, 2, SC], BF16, tag="trig")
            cpack = cp.tile([P, 259], BF16, tag="cpack")
            mnegc = cp.tile([1, SC + P], BF16, tag="mnegc")
            bkq = cp.tile([P, 2 * FT], F32, tag="bkq")
            bkT = bkq[:, 0:FT]
            bqT = bkq[:, FT:2 * FT]
            bvb = cp.tile([1, DM], BF16, tag="bvb")
            phiq_all = cp.tile([P, FT, SC], BF16, tag="phiq_all")

            # ---- wave 1 loads: big K-path operands on the SP/HWDGE queue,
            # small tables on the Pool/SWDGE queue (parallel issue paths) ----
            nc.sync.dma_start(wkT[:, 0:1], wkT_in[:, 0:1])
            nc.sync.dma_start(xT[:, 0, 0:4], xT_in[:, 0, 0:4])
            nc.sync.dma_start(xT[:, 0, 4:KT], xT_in[:, 0, 4:KT])
            nc.sync.dma_start(wkT[:, 1:4], wkT_in[:, 1:4])
            nc.sync.dma_start(wkT[:, 4:FT], wkT_in[:, 4:FT])
            nc.sync.dma_start(xT[:, 1, 0:4], xT_in[:, 1, 0:4])
            nc.sync.dma_start(xT[:, 1, 4:KT], xT_in[:, 1, 4:KT])
            nc.gpsimd.dma_start(trig[:, :, 0:1024], trig_in[:, :, 0:1024])
            nc.gpsimd.dma_start(bkq[:], bkq_in)
            nc.gpsimd.dma_start(cpack[:], cpack_in)
            nc.gpsimd.dma_start(mnegc[:], mnegc_in)
            nc.gpsimd.dma_start(trig[:, :, 1024:SC], trig_in[:, :, 1024:SC])
            # preload the Exp activation table while ACT is otherwise idle
            actwarm = cp.tile([1, 2], BF16, tag="actwarm")
            nc.scalar.activation(actwarm[:], trig[0:1, 0, 0:2], AF.Exp)

            # accumulators
            ksum_parts = cp.tile([P, FT * NJ], F32, tag="ksum_parts")
            psk_nat = cp.tile([P, NST, H], BF16, tag="psk_nat")
            zqext = cp.tile([H + 1, SC], BF16, tag="zqext")
            nc.vector.memset(zqext[:], 1.0)
            wo2ext = cp.tile([H + 1, DM], BF16, tag="wo2ext")
            nc.gpsimd.dma_start(wo2ext[H:H + 1, :], bob_in)

            # ---- wave 2 loads (Pool/SWDGE queue), emitted mid-K-path.
            # The scheduler reorders independent queue entries, so gate each
            # transfer behind an early K-path product with a real WAW dep
            # (corner-copy into the dst).
            def wave2():
                def gated(corner, dst, src):
                    nc.gpsimd.tensor_copy(corner, psk_nat[0:2, 4, 0:2])
                    nc.gpsimd.dma_start(dst, src)

                gated(wqT[0:2, 0, 0, 0:2], wqT[:], wqT_in)
                for sg in range(4):
                    gated(
                        xn[0:2, 4 * sg, 0:2],
                        xn[:, 4 * sg:4 * sg + 4], xn_in[:, 4 * sg:4 * sg + 4]
                    )
                gated(wvT[0:2, 0, 0:2], wvT[:], wvT_in)
                gated(woT[0:2, 0, 0:2], woT[:], woT_in)
                nc.gpsimd.dma_start(bvb[:], bvb_in)

            def unit_front(path, t, j, s12_pool=False):
                """proj chain + bias evac + cos/sin products, 512-wide."""
                jsl = slice(j * 512, (j + 1) * 512)
                J, jh = j // 2, (j % 2) * 512
                wT = wkT if path == "k" else wqT
                bofs = 0 if path == "k" else FT
                projP = pb.tile([P, 512], F32, tag="pb")
                if PROJ_FP8:
                    for kp in range(KT // 2):
                        nc.tensor.matmul(
                            projP[:], wT[:, t, 2 * kp:2 * kp + 2],
                            xT[:, J, 2 * kp:2 * kp + 2, jh:jh + 512],
                            start=(kp == 0), stop=(kp == KT // 2 - 1),
                            perf_mode=mybir.MatmulPerfMode.DoubleRow,
                        )
                else:
                    for kt in range(KT):
                        nc.tensor.matmul(
                            projP[:], wT[:, t, kt], xT[:, J, kt, jh:jh + 512],
                            start=(kt == 0), stop=(kt == KT - 1),
                        )
                ck = wp.tile([P, 1, 512], BF16, tag="ck")
                nc.scalar.activation(
                    ck[:, 0], projP[:], AF.Identity,
                    bias=bkq[:, bofs + t:bofs + t + 1],
                )
                # s12[:,0,:] = ck*cos, s12[:,1,:] = ck*sin in one op via a
                # stride-0 broadcast of ck over the middle dim
                s12 = wp.tile([P, 2, 512], BF16, tag="s12")
                eng = nc.gpsimd if s12_pool else nc.vector
                eng.tensor_tensor(
                    s12[:], ck[:].broadcast_to([P, 2, 512]), trig[:, :, jsl],
                    ALU.mult,
                )
                return (path, t, j, projP, s12)

            def unit_back(state, out_phi, stt_pool=False, relu_act=True):
                """rope (reusing the proj PSUM slot) + feature map."""
                path, t, j, projP, s12 = state
                nc.tensor.matmul(
                    projP[:], cpack[:, 0:P], s12[:, 0], start=True, stop=False
                )
                nc.tensor.matmul(
                    projP[:], cpack[:, P:2 * P], s12[:, 1],
                    start=False, stop=(path == "q"),
                )
                if path == "k":
                    nc.tensor.matmul(
                        projP[:], mnegc[0:1, SC:SC + P],
                        mnegc[0:1, j * 512:(j + 1) * 512],
                        start=False, stop=True,
                    )
                e = wp.tile([P, 512], BF16, tag="e")
                nc.scalar.activation(e[:], projP[:], AF.Exp)
                r = wp.tile([P, 512], BF16, tag="r")
                if path == "k":
                    # phi = min(e,1) + r with the ksum accumulation riding on
                    # the scalar_tensor_tensor
                    if relu_act:
                        nc.scalar.activation(r[:], projP[:], AF.Relu)
                    else:
                        nc.vector.tensor_scalar_max(r[:], projP[:], 0.0)
                    nc.vector.scalar_tensor_tensor(
                        out_phi, e[:], 1.0, r[:], ALU.min, ALU.add,
                        accum_out=ksum_parts[:, t * NJ + j:t * NJ + j + 1],
                    )
                else:
                    # phi = min(exp(v), relu(v)+1) exactly; the min runs in
                    # the DVE 2x bf16 mode
                    nc.vector.tensor_scalar(
                        r[:], projP[:], 0.0, 1.0, ALU.max, ALU.add
                    )
                    nc.vector.tensor_tensor(out_phi, e[:], r[:], ALU.min)

            # ============ K path: software-pipelined, j-outer ============
            def psk_stage(phik, t, j):
                pskT = pp_s.tile([P, 8], F32, tag="small")
                for sub in range(4):
                    nc.tensor.matmul(
                        pskT[:, 2 * sub:2 * sub + 2],
                        phik[:, sub * P:(sub + 1) * P],
                        cpack[:, 2 * P:2 * P + 2],
                    )
                nc.vector.tensor_copy(
                    psk_nat[:, 4 * j:4 * j + 4, 2 * t:2 * t + 2],
                    pskT.rearrange("p (st hh) -> p st hh", hh=2),
                )

            def k_back(state):
                path, t, j, _, _ = state
                phik = wp.tile([P, 512], BF16, tag="phik")
                unit_back(state, phik[:], relu_act=(t % 2 == 0))
                psk_stage(phik, t, j)

            import collections as _c
            kq = _c.deque()
            for j in range(NJ):
                for t in range(FT):
                    kq.append(unit_front("k", t, j, s12_pool=(t % 4 == 3 and j > 0)))
                    if len(kq) > 2:
                        st = kq.popleft()
                        k_back(st)
                        if st[1:3] == (0, 1):
                            wave2()
            qpend = unit_front("q", 0, 0, s12_pool=False)
            while kq:
                k_back(kq.popleft())

            # ksum_flat = sum of the 4 j-chunks
            kv4 = ksum_parts.rearrange("p (t j) -> p t j", j=NJ)
            kst1 = cp.tile([P, FT], F32, tag="kst1")
            kst2 = cp.tile([P, FT], F32, tag="kst2")
            ksum_flat = cp.tile([P, FT], F32, tag="ksum_flat")
            nc.vector.tensor_tensor(kst1[:], kv4[:, :, 0], kv4[:, :, 1], ALU.add)
            nc.vector.tensor_tensor(kst2[:], kv4[:, :, 2], kv4[:, :, 3], ALU.add)
            nc.vector.tensor_tensor(ksum_flat[:], kst1[:], kst2[:], ALU.add)

            # xk in transposed form xkT[p, kt, h] = sum_s x[s, kt*128+p] psk[s, h]
            xkP = pp_s.tile([P, KT * H], F32, tag="small")
            xkv = xkP.rearrange("p (kt h) -> p kt h", h=H)
            for kt in range(KT):
                for st in range(NST):
                    nc.tensor.matmul(
                        xkv[:, kt], xn[:, st, kt * P:(kt + 1) * P],
                        psk_nat[:, st, :],
                        start=(st == 0), stop=(st == NST - 1),
                    )
            xk_sb = cp.tile([P, KT * H], F32, tag="xk_sb")
            nc.vector.tensor_copy(xk_sb[:], xkP[:])
            # psktot = colsum of psk
            ptP = pp_s.tile([1, H], F32, tag="small")
            for st in range(NST):
                nc.tensor.matmul(
                    ptP[:], cpack[:, 2 * P + 2:2 * P + 3], psk_nat[:, st, :],
                    start=(st == 0), stop=(st == NST - 1),
                )
            psktot_f = cp.tile([1, H], F32, tag="psktot_f")
            nc.vector.tensor_copy(psktot_f[:], ptP[:])

            # ============ collective (pairs share a batch) ============
            with nc.allow_non_contiguous_dma(reason="70KB collective bundle"):
                nc.sync.dma_start(
                    cc_i[0:CC_XK].rearrange("(a b) -> a b", a=P), xk_sb[:]
                )
                nc.sync.dma_start(
                    cc_i[CC_XK:CC_XK + CC_KS].rearrange("(a b) -> a b", a=P),
                    ksum_flat[:],
                )
                nc.sync.dma_start(
                    cc_i[CC_XK + CC_KS:CC_LEN].rearrange("(a b) -> a b", a=1),
                    psktot_f[:],
                )
            if collective:
                nc.gpsimd.collective_compute(
                    "AllReduce",
                    ALU.add,
                    replica_groups=[[0, 1], [2, 3], [4, 5], [6, 7]],
                    ins=[cc_i.opt()],
                    outs=[cc_o.opt()],
                )
            else:  # timing-model variant: TimelineSim can't model collectives
                nc.sync.dma_start(cc_o[:], cc_i[:])

            # ============ Q path: phi production (collective-independent) ==
            def q_front(t, j):
                return unit_front("q", t, j, s12_pool=(t % 2 == 1))

            def q_back(state):
                _, t, j, _, _ = state
                unit_back(
                    state, phiq_all[:, t, j * 512:(j + 1) * 512],
                    relu_act=(t % 4 == 2),
                )

            qq = _c.deque([qpend])
            for j in range(0, 2):
                for t in range(FT):
                    if j == 0 and t == 0:
                        continue
                    qq.append(q_front(t, j))
                    if len(qq) > 2:
                        q_back(qq.popleft())

            # ---- collective consumers: emitted ~2 j-chunks (35us) after the
            # all-reduce was issued, so even a slow collective is off the
            # critical path by the time these hit the engine queues ----
            xk_r = cp.tile([P, KT * H], F32, tag="xk_r")
            ksum_r = cp.tile([P, FT], F32, tag="ksum_r")
            psktot_r = cp.tile([1, H], F32, tag="psktot_r")
            with nc.allow_non_contiguous_dma(reason="70KB collective bundle"):
                nc.sync.dma_start(
                    xk_r[:], cc_o[0:CC_XK].rearrange("(a b) -> a b", a=P)
                )
                nc.sync.dma_start(
                    ksum_r[:],
                    cc_o[CC_XK:CC_XK + CC_KS].rearrange("(a b) -> a b", a=P),
                )
                nc.sync.dma_start(
                    psktot_r[:],
                    cc_o[CC_XK + CC_KS:CC_LEN].rearrange("(a b) -> a b", a=1),
                )
            xkT8 = cp.tile([P, KT, H], BF16, tag="xkT8")
            nc.vector.tensor_copy(
                xkT8[:], xk_r.rearrange("p (kt h) -> p kt h", h=H)
            )
            psktot_rb = cp.tile([1, H], BF16, tag="psktot_rb")
            nc.vector.tensor_copy(psktot_rb[:], psktot_r[:])
            # qones[p, m, t]: m 0/1 = head-half ones, m 2/3 = ksum halves
            qones = cp.tile([P, 4, FT], BF16, tag="qones")
            nc.vector.memset(qones[:], 0.0)
            nc.vector.memset(qones[0:64, 0, :], 1.0)
            nc.vector.memset(qones[64:P, 1, :], 1.0)
            nc.vector.tensor_copy(qones[0:64, 2, :], ksum_r[0:64, :])
            nc.vector.tensor_copy(qones[64:P, 3, :], ksum_r[64:P, :])

            # kvsum / Wo2
            kvsum_f = cp.tile([P, FT], F32, tag="kvsum_f")
            for t in range(FT):
                tsl = slice(t * P, (t + 1) * P)
                kvP = pp_s.tile([P, H], F32, tag="small")
                for kt in range(KT):
                    nc.tensor.matmul(
                        kvP[:], wvT[:, kt, tsl], xkT8[:, kt, :],
                        start=(kt == 0), stop=False,
                    )
                nc.tensor.matmul(
                    kvP[:], bvb[:, tsl], psktot_rb[:],
                    start=False, stop=True,
                )
                nc.vector.tensor_copy(
                    kvsum_f[0:64, t:t + 1], kvP[0:64, 2 * t:2 * t + 1]
                )
                nc.vector.tensor_copy(
                    kvsum_f[64:P, t:t + 1], kvP[64:P, 2 * t + 1:2 * t + 2]
                )
            kvsel = cp.tile([P, KT, H], BF16, tag="kvsel")
            nc.vector.memset(kvsel[:], 0.0)
            kvselv = kvsel.rearrange("p kt h -> p (kt h)")
            # element (t, 2t [+1]) of the [8,16] grid = flat index 18t [+1]
            nc.vector.tensor_copy(
                kvselv[0:64, 0:127:18], kvsum_f[0:64, :]
            )
            nc.vector.tensor_copy(
                kvselv[64:P, 1:128:18], kvsum_f[64:P, :]
            )
            for half in range(2):
                hsl = slice(half * 512, (half + 1) * 512)
                w2P = pp_s.tile([H, 512], F32, tag="small")
                for kt in range(KT):
                    nc.tensor.matmul(
                        w2P[:], kvsel[:, kt, :], woT[:, kt, hsl],
                        start=(kt == 0), stop=(kt == KT - 1),
                    )
                nc.scalar.copy(wo2ext[0:H, hsl], w2P[:])

            # ============ qd/z/y tail for one j-chunk ====================
            qd_nat = cp.tile([P, NST * FT * 4], F32, tag="qd_nat")
            qdv = qd_nat.rearrange("p (st t m) -> p st t m", st=NST, t=FT)
            den_cl = cp.tile([P, 256], F32, tag="den_cl")
            zr = cp.tile([P, 256], F32, tag="zr")
            zq_c = cp.tile([P, 256], BF16, tag="zq_c")
            zqv = zq_c.rearrange("p (st t hh) -> p st t hh", st=NST, t=FT)
            zrv = zr.rearrange("p (st t hh) -> p st t hh", st=NST, t=FT)
            dclv = den_cl.rearrange("p (st t hh) -> p st t hh", st=NST, t=FT)

            def tail_qd(j):
                qdP = pp_s.tile([P, FT * 16], F32, tag="small")
                qdPv = qdP.rearrange("p (t s m) -> p t s m", t=FT, s=4)
                for t in range(FT):
                    for sub in range(4):
                        st = 4 * j + sub
                        nc.tensor.matmul(
                            qdPv[:, t, sub, :],
                            phiq_all[:, t, st * P:(st + 1) * P],
                            qones[:, :, t],
                        )
                sts = slice(4 * j, 4 * (j + 1))
                nc.vector.tensor_copy(
                    qdv[:, sts, :, :],
                    qdP.rearrange("p (t s m) -> p s t m", t=FT, s=4),
                )

            def tail_z(j):
                sts = slice(4 * j, 4 * (j + 1))
                zsl = slice(64 * j, 64 * (j + 1))
                sts = slice(4 * j, 4 * (j + 1))
                nc.vector.tensor_scalar_max(
                    dclv[:, sts], qdv[:, sts, :, 2:4], EPS
                )
                nc.vector.reciprocal(zr[:, zsl], den_cl[:, zsl])
                nc.vector.tensor_tensor(
                    zqv[:, sts], zrv[:, sts], qdv[:, sts, :, 0:2], ALU.mult
                )
                for sub in range(4):
                    st = 4 * j + sub
                    ssl = slice(st * P, (st + 1) * P)
                    zP = pp_s.tile([H, P], BF16, tag="small")
                    nc.tensor.transpose(
                        zP[:], zq_c[:, st * H:(st + 1) * H], cpack[:, 0:P]
                    )
                    nc.scalar.copy(zqext[0:H, ssl], zP[:])

            def tail_y(j):
                for sub in range(4):
                    st = 4 * j + sub
                    ssl = slice(st * P, (st + 1) * P)
                    ysb = yp.tile([P, DM], F32, tag="ysb")
                    for half in range(2):
                        hsl = slice(half * 512, (half + 1) * 512)
                        yP = pp_s.tile([P, 512], F32, tag="small")
                        nc.tensor.matmul(yP[:], zqext[:, ssl], wo2ext[:, hsl])
                        if half == 0:
                            nc.vector.tensor_copy(ysb[:, hsl], yP[:])
                        else:
                            nc.scalar.copy(ysb[:, hsl], yP[:])
                    nc.sync.dma_start(y_out[ssl, :], ysb[:])

            def tail(j):
                tail_qd(j)
                tail_z(j)
                tail_y(j)

            tail(0)
            for t in range(FT):
                qq.append(q_front(t, 2))
                q_back(qq.popleft())
            tail(1)
            for t in range(0, 4):
                qq.append(q_front(t, 3))
                q_back(qq.popleft())
            tail_qd(2)
            for t in range(4, FT):
                qq.append(q_front(t, 3))
                q_back(qq.popleft())
            while qq:
                q_back(qq.popleft())
            tail_z(2)
            tail_qd(3)
            tail_y(2)
            tail_z(3)
            tail_y(3)

    nc._dbg = dict(
        psk_nat=psk_nat, ksum_parts=ksum_parts, ksum_flat=ksum_flat,
        xk_sb=xk_sb, psktot_f=psktot_f, xkT8=xkT8, ksum_r=ksum_r,
        kvsum_f=kvsum_f, kvsel=kvsel, wo2ext=wo2ext, qones=qones,
        qd_nat=qd_nat, zq_c=zq_c, phiq_all=phiq_all, zqext=zqext,
    )
    nc.finalize()
    return nc


def _consts():
    psign = np.zeros((P, P), np.float32)
    for h in range(2):
        for i in range(32):
            psign[h * 64 + 32 + i, h * 64 + i] = -1.0   # even' = .. - s*odd
            psign[h * 64 + i, h * 64 + 32 + i] = 1.0    # odd'  = .. + s*even
    ones3 = np.zeros((P, 3), np.float32)
    ones3[0:64, 0] = 1.0
    ones3[64:P, 1] = 1.0
    ones3[:, 2] = 1.0
    cpack = np.concatenate(
        [np.eye(P, dtype=np.float32), psign, ones3], axis=1
    ).astype(bf)
    return {"cpack": np.ascontiguousarray(cpack)}


def _perm_rows(w):
    """Per-head [even|odd] row permutation: dest row h*64+pr*32+i comes from
    source row h*64+2i+pr."""
    return np.ascontiguousarray(
        w.reshape(H, 32, 2, -1).transpose(0, 2, 1, 3).reshape(DM, -1)
    )


@functools.lru_cache(maxsize=1)
def _program():
    return build_program()


def make_in_maps(inputs):
    consts = _consts()
    f32 = np.float32

    def wT_tmajor(w):  # [128, t, kt, 128] from permuted W
        a = np.ascontiguousarray(_perm_rows(np.asarray(w, f32)).T.astype(pdt))
        return np.ascontiguousarray(
            a.reshape(KT, P, FT, P).transpose(1, 2, 0, 3)
        )

    def wT_plain(w):   # [128, kt, 1024] = W.T tiles
        a = np.ascontiguousarray(np.asarray(w, f32).T.astype(bf))
        return np.ascontiguousarray(a.reshape(KT, P, DM).transpose(1, 0, 2))

    def bT_perm(b):    # [128, t] f32 permuted bias columns
        bp = np.asarray(b, f32).reshape(H, 32, 2).transpose(0, 2, 1).reshape(DM)
        return np.ascontiguousarray(bp.reshape(FT, P).T)

    bkq = np.concatenate(
        [bT_perm(inputs["bk"]), bT_perm(inputs["bq"])], axis=1
    )
    shared = {
        "wqT": wT_tmajor(inputs["Wq"]),
        "wkT": wT_tmajor(inputs["Wk"]),
        "wvT": wT_plain(inputs["Wv"]),
        "woT": wT_plain(inputs["Wo"]),
        "bkq": np.ascontiguousarray(bkq),
        "bvb": np.asarray(inputs["bv"], f32).astype(bf).reshape(1, DM),
        "bob": np.asarray(inputs["bo"], f32).astype(bf).reshape(1, DM),
        **consts,
    }
    x = np.asarray(inputs["x"], f32)
    pos = np.asarray(inputs["rotary_pos_enc"], f32)
    mask = np.asarray(inputs["padding_mask"], np.int32)
    # cos/sin broadcast tables per seq half: trig[:,0,:]=cos, [:,1,:]=sin
    trig = []
    for hf in range(2):
        sl = slice(hf * SC, (hf + 1) * SC)
        tt = np.zeros((P, 2, SC), bf)
        tt[:, 0, :] = np.tile(pos[sl, 0, 32:64].T.astype(bf), (4, 1))
        tt[:, 1, :] = np.tile(pos[sl, 0, 0:32].T.astype(bf), (4, 1))
        trig.append(np.ascontiguousarray(tt))
    in_maps = []
    for c in range(N_CORES):
        b_, hf = c // 2, c % 2
        sl = slice(hf * SC, (hf + 1) * SC)
        xc = np.ascontiguousarray(x[b_, sl])                     # [2048,1024]
        xTc = np.ascontiguousarray(
            xc.T.astype(pdt).reshape(KT, P, 2, 1024).transpose(1, 2, 0, 3)
        )
        xnc = np.ascontiguousarray(
            xc.astype(bf).reshape(NST, P, DM).transpose(1, 0, 2)
        )
        mnegc = np.ones((1, SC + P), np.float32)
        mnegc[0, 0:SC] = mask[b_, sl].astype(f32) * -1e4
        in_maps.append(
            {
                "xT": xTc,
                "xn": xnc,
                "trig": trig[hf],
                "mnegc": np.ascontiguousarray(mnegc.astype(bf)),
                **shared,
            }
        )
    return in_maps


def run(inputs, **kwargs):
    nc = _program()
    in_maps = make_in_maps(inputs)
    res = run_bass_kernel_spmd(
        nc, in_maps, core_ids=list(range(N_CORES)), **kwargs
    )
    out = np.zeros((B, S, DM), np.float32)
    for c in range(N_CORES):
        b_, hf = c // 2, c % 2
        out[b_, hf * SC:(hf + 1) * SC, :] = res.results[c]["y"]
    return out, res


def kernel(**inputs) -> np.ndarray:
    out, _ = run(inputs)
    return out


# revision 75
# speedup vs baseline: 1.0506x; 1.0246x over previous
"""Trainium2 Bass kernel for nn_MultiHeadAttention_27711128994021.

Reference math (faithful to the oracle, including its independent-sum einsum):
  q = x@Wq.T+bq ; k = x@Wk.T+bk ; v = x@Wv.T+bv       (B,S,H,D)
  rq, rk = rope(pos, q, k)
  phi_q = elu(rq)+1 ; phi_k = (elu(rk)+1) * notpad
  attn[b,s,h,v] = z[b,h,s] * (sum_q phi_q[b,s,h,q]) * (sum_k kv[b,h,v,k])
    with kv = einsum("bshv,bshk->bhvk", v, phi_k), z = 1/clip(phi_q . k_sum)
  out = attn @ Wo.T + bo

Because q and k are summed independently in the attn einsum, attention is
rank-1 per (b,h):  attn = zq[s,h] * kvsum[h,v]  where
  kvsum[h,v] = sum_s v[s,hv] * psk[s,h],  psk = rowsum(phi_k)
so the V projection collapses:  kvsum = (Wv @ (psk.T @ x).T)_head-diag  and
the out projection collapses to rank-16:  y = zq @ Wo2 + bo with
  Wo2[h,n] = sum_v kvsum[h,v] Wo[n,64h+v].
Only the Q and K projections remain as large matmuls.

Sharding: 8 cores = (batch b, seq half). Per core: 2048 rows of one batch.
Cross-core data: all-reduce (pairs) of xk=psk.T@x [16,1024], k_sum [1024],
psktot [16] — 70KB, hidden behind the Q-path phi production.

All operand preparation (bf16 casts, transposes, per-head [even|odd] row
permutation of Wq/Wk, cos/sin broadcast tables, permuted bias columns) is
done host-side in numpy; the device only streams compute.

Padding mask is applied ADDITIVELY: a rank-1 matmul adds -1e4 to every
masked key column inside the rope PSUM accumulation, so
phi_k = min(exp(v),1)+relu(v) (identical formula to phi_q) comes out zero
for masked rows. elu(x)+1 == min(exp(x),1) + max(x,0) exactly.

phi_q is stashed fully in SBUF so its production has no dependency on the
collective; the qd/z/y tail consumes it j-chunk by j-chunk afterwards,
giving the all-reduce ~35us of slack off the critical path.

Schedule: Q/K projections run as fp8e4m3 DoubleRow matmuls (4 chained
k-pair passes); each (t, j) unit is software-pipelined two deep
(proj+evac+s12 fronts run ahead of rope+exp+relu+phi backs), with rope
accumulating into the *same* PSUM slot the projection used, so one
1-bank PSUM slot per unit gives six units in flight. Work is spread
across engines: ACT does evac/exp (+some relu), DVE does s12/relu/phi
(min,add with ksum accum), the Pool engine takes a quarter of the
s12 products (it cannot run TensorScalarPtr or touch PSUM on real HW).
Weight/x loads stream in two waves (SP/HWDGE for big K operands,
Pool/SWDGE for tables), with wave 2 gated behind an early K product to
keep the shared DMA engines clear during ramp.
"""

import functools

import numpy as np
import ml_dtypes

import concourse.bass as bass
import concourse.mybir as mybir
import concourse.tile as tile
from concourse import bacc
from concourse.bass_utils import run_bass_kernel_spmd

F32 = mybir.dt.float32
BF16 = mybir.dt.bfloat16
FP8 = mybir.dt.float8e4
I32 = mybir.dt.int32
PROJ_FP8 = True   # fp8e4m3 + DoubleRow for the Q/K projections
PDT = FP8 if PROJ_FP8 else BF16
AF = mybir.ActivationFunctionType
ALU = mybir.AluOpType

P = 128
B, S, H, D = 4, 4096, 16, 64
DM = H * D            # 1024
SC = 2048             # seq rows per core
KT = DM // P          # 8 k tiles
FT = DM // P          # 8 feature tiles (2 heads each)
NJ = SC // 512        # 4 s-chunks of 512
NST = SC // P         # 16 seq tiles of 128
EPS = 1e-6
N_CORES = 8
CC_XK, CC_KS, CC_PT = P * KT * H, P * FT, 16   # collective bundle sections
CC_LEN = CC_XK + CC_KS + CC_PT

bf = ml_dtypes.bfloat16
f8 = ml_dtypes.float8_e4m3fn
pdt = f8 if PROJ_FP8 else bf


def build_program(collective=True):
    nc = bacc.Bacc(
        "TRN2", target_bir_lowering=False, debug=False, num_devices=N_CORES
    )

    # ---- I/O (everything already laid out / cast host-side) ----
    xT_in = nc.dram_tensor("xT", [P, 2, KT, 1024], PDT, kind="ExternalInput").ap()
    xn_in = nc.dram_tensor("xn", [P, NST, DM], BF16, kind="ExternalInput").ap()
    wqT_in = nc.dram_tensor("wqT", [P, FT, KT, P], PDT, kind="ExternalInput").ap()
    wkT_in = nc.dram_tensor("wkT", [P, FT, KT, P], PDT, kind="ExternalInput").ap()
    wvT_in = nc.dram_tensor("wvT", [P, KT, DM], BF16, kind="ExternalInput").ap()
    woT_in = nc.dram_tensor("woT", [P, KT, DM], BF16, kind="ExternalInput").ap()
    # trig[:,0,:]=cos, [:,1,:]=sin ; cpack = [ident|psign|ones3] ;
    # mnegc = [-1e4*mask | ones-row] ; bkq = [bkT|bqT]
    trig_in = nc.dram_tensor("trig", [P, 2, SC], BF16, kind="ExternalInput").ap()
    cpack_in = nc.dram_tensor("cpack", [P, 259], BF16, kind="ExternalInput").ap()
    mnegc_in = nc.dram_tensor("mnegc", [1, SC + P], BF16, kind="ExternalInput").ap()
    bkq_in = nc.dram_tensor("bkq", [P, 2 * FT], F32, kind="ExternalInput").ap()
    bvb_in = nc.dram_tensor("bvb", [1, DM], BF16, kind="ExternalInput").ap()
    bob_in = nc.dram_tensor("bob", [1, DM], BF16, kind="ExternalInput").ap()
    y_out = nc.dram_tensor("y", [SC, DM], F32, kind="ExternalOutput").ap()

    with tile.TileContext(nc) as tc:
        with (
            tc.tile_pool(name="const", bufs=1) as cp,
            tc.tile_pool(name="work", bufs=5 if PROJ_FP8 else 2) as wp,
            tc.tile_pool(name="ypool", bufs=3) as yp,
            tc.tile_pool(name="pbig", bufs=6, space="PSUM") as pb,
            tc.tile_pool(name="psmall", bufs=2, space="PSUM") as pp_s,
            tc.tile_pool(name="dram", bufs=1, space="DRAM") as dp,
        ):
            cc_i = dp.tile([CC_LEN], F32, tag="cc_i")
            cc_o = dp.tile([CC_LEN], F32, tag="cc_o")

            # ---- SBUF residents ----
            xT = cp.tile([P, 2, KT, 1024], PDT, tag="xT")
            xn = cp.tile([P, NST, DM], BF16, tag="xn")
            wqT = cp.tile([P, FT, KT, P], PDT, tag="wqT")
            wkT = cp.tile([P, FT, KT, P], PDT, tag="wkT")
            wvT = cp.tile([P, KT, DM], BF16, tag="wvT")
            woT = cp.tile([P, KT, DM], BF16, tag="woT")
            trig = cp.tile([P, 2, SC], BF16, tag="trig")
            cpack = cp.tile([P, 259], BF16, tag="cpack")
            mnegc = cp.tile([1, SC + P], BF16, tag="mnegc")
            bkq = cp.tile([P, 2 * FT], F32, tag="bkq")
            bkT = bkq[:, 0:FT]
            bqT = bkq[:, FT:2 * FT]
            bvb = cp.tile([1, DM], BF16, tag="bvb")
            phiq_all = cp.tile([P, FT, SC], BF16, tag="phiq_all")

            # ---- wave 1 loads: big K-path operands on the SP/HWDGE queue,
            # small tables on the Pool/SWDGE queue (parallel issue paths) ----
            nc.sync.dma_start(wkT[:, 0:1], wkT_in[:, 0:1])
            nc.sync.dma_start(xT[:, 0, 0:2], xT_in[:, 0, 0:2])
            nc.sync.dma_start(xT[:, 0, 2:4], xT_in[:, 0, 2:4])
            nc.sync.dma_start(xT[:, 0, 4:KT], xT_in[:, 0, 4:KT])
            nc.sync.dma_start(wkT[:, 1:4], wkT_in[:, 1:4])
            nc.sync.dma_start(wkT[:, 4:FT], wkT_in[:, 4:FT])
            nc.sync.dma_start(xT[:, 1, 0:4], xT_in[:, 1, 0:4])
            nc.sync.dma_start(xT[:, 1, 4:KT], xT_in[:, 1, 4:KT])
            nc.gpsimd.dma_start(trig[:, :, 0:1024], trig_in[:, :, 0:1024])
            nc.gpsimd.dma_start(bkq[:], bkq_in)
            nc.gpsimd.dma_start(cpack[:], cpack_in)
            nc.gpsimd.dma_start(mnegc[:], mnegc_in)
            nc.gpsimd.dma_start(trig[:, :, 1024:SC], trig_in[:, :, 1024:SC])
            # preload the Exp activation table while ACT is otherwise idle
            actwarm = cp.tile([1, 2], BF16, tag="actwarm")
            nc.scalar.activation(actwarm[:], trig[0:1, 0, 0:2], AF.Exp)

            # accumulators
            ksum_parts = cp.tile([P, FT * NJ], F32, tag="ksum_parts")
            psk_nat = cp.tile([P, NST, H], BF16, tag="psk_nat")
            zqext = cp.tile([H + 1, SC], BF16, tag="zqext")
            nc.vector.memset(zqext[:], 1.0)
            wo2ext = cp.tile([H + 1, DM], BF16, tag="wo2ext")
            nc.gpsimd.dma_start(wo2ext[H:H + 1, :], bob_in)

            # ---- wave 2 loads (Pool/SWDGE queue), emitted mid-K-path.
            # The scheduler reorders independent queue entries, so gate each
            # transfer behind an early K-path product with a real WAW dep
            # (corner-copy into the dst).
            def wave2():
                def gated(corner, dst, src):
                    nc.gpsimd.tensor_copy(corner, psk_nat[0:2, 4, 0:2])
                    nc.gpsimd.dma_start(dst, src)

                gated(wqT[0:2, 0, 0, 0:2], wqT[:], wqT_in)
                for sg in range(4):
                    gated(
                        xn[0:2, 4 * sg, 0:2],
                        xn[:, 4 * sg:4 * sg + 4], xn_in[:, 4 * sg:4 * sg + 4]
                    )
                gated(wvT[0:2, 0, 0:2], wvT[:], wvT_in)
                gated(woT[0:2, 0, 0:2], woT[:], woT_in)
                nc.gpsimd.dma_start(bvb[:], bvb_in)

            def unit_front(path, t, j, s12_pool=False):
                """proj chain + bias evac + cos/sin products, 512-wide."""
                jsl = slice(j * 512, (j + 1) * 512)
                J, jh = j // 2, (j % 2) * 512
                wT = wkT if path == "k" else wqT
                bofs = 0 if path == "k" else FT
                projP = pb.tile([P, 512], F32, tag="pb")
                if PROJ_FP8:
                    for kp in range(KT // 2):
                        nc.tensor.matmul(
                            projP[:], wT[:, t, 2 * kp:2 * kp + 2],
                            xT[:, J, 2 * kp:2 * kp + 2, jh:jh + 512],
                            start=(kp == 0), stop=(kp == KT // 2 - 1),
                            perf_mode=mybir.MatmulPerfMode.DoubleRow,
                        )
                else:
                    for kt in range(KT):
                        nc.tensor.matmul(
                            projP[:], wT[:, t, kt], xT[:, J, kt, jh:jh + 512],
                            start=(kt == 0), stop=(kt == KT - 1),
                        )
                ck = wp.tile([P, 1, 512], BF16, tag="ck")
                nc.scalar.activation(
                    ck[:, 0], projP[:], AF.Identity,
                    bias=bkq[:, bofs + t:bofs + t + 1],
                )
                # s12[:,0,:] = ck*cos, s12[:,1,:] = ck*sin in one op via a
                # stride-0 broadcast of ck over the middle dim
                s12 = wp.tile([P, 2, 512], BF16, tag="s12")
                eng = nc.gpsimd if s12_pool else nc.vector
                eng.tensor_tensor(
                    s12[:], ck[:].broadcast_to([P, 2, 512]), trig[:, :, jsl],
                    ALU.mult,
                )
                return (path, t, j, projP, s12)

            def unit_back(state, out_phi, stt_pool=False, relu_act=True):
                """rope (reusing the proj PSUM slot) + feature map."""
                path, t, j, projP, s12 = state
                nc.tensor.matmul(
                    projP[:], cpack[:, 0:P], s12[:, 0], start=True, stop=False
                )
                nc.tensor.matmul(
                    projP[:], cpack[:, P:2 * P], s12[:, 1],
                    start=False, stop=(path == "q"),
                )
                if path == "k":
                    nc.tensor.matmul(
                        projP[:], mnegc[0:1, SC:SC + P],
                        mnegc[0:1, j * 512:(j + 1) * 512],
                        start=False, stop=True,
                    )
                e = wp.tile([P, 512], BF16, tag="e")
                nc.scalar.activation(e[:], projP[:], AF.Exp)
                r = wp.tile([P, 512], BF16, tag="r")
                if path == "k":
                    # phi = min(e,1) + r with the ksum accumulation riding on
                    # the scalar_tensor_tensor
                    if relu_act:
                        nc.scalar.activation(r[:], projP[:], AF.Relu)
                    else:
                        nc.vector.tensor_scalar_max(r[:], projP[:], 0.0)
                    nc.vector.scalar_tensor_tensor(
                        out_phi, e[:], 1.0, r[:], ALU.min, ALU.add,
                        accum_out=ksum_parts[:, t * NJ + j:t * NJ + j + 1],
                    )
                else:
                    # phi = min(exp(v), relu(v)+1) exactly; the min runs in
                    # the DVE 2x bf16 mode
                    nc.vector.tensor_scalar(
                        r[:], projP[:], 0.0, 1.0, ALU.max, ALU.add
                    )
                    nc.vector.tensor_tensor(out_phi, e[:], r[:], ALU.min)

            # ============ K path: software-pipelined, j-outer ============
            def psk_stage(phik, t, j):
                pskT = pp_s.tile([P, 8], F32, tag="small")
                for sub in range(4):
                    nc.tensor.matmul(
                        pskT[:, 2 * sub:2 * sub + 2],
                        phik[:, sub * P:(sub + 1) * P],
                        cpack[:, 2 * P:2 * P + 2],
                    )
                nc.vector.tensor_copy(
                    psk_nat[:, 4 * j:4 * j + 4, 2 * t:2 * t + 2],
                    pskT.rearrange("p (st hh) -> p st hh", hh=2),
                )

            def k_back(state):
                path, t, j, _, _ = state
                phik = wp.tile([P, 512], BF16, tag="phik")
                unit_back(state, phik[:], relu_act=(t % 2 == 0))
                psk_stage(phik, t, j)

            import collections as _c
            kq = _c.deque()
            for j in range(NJ):
                for t in range(FT):
                    kq.append(unit_front("k", t, j, s12_pool=(t % 4 == 3 and j > 0)))
                    if len(kq) > 2:
                        st = kq.popleft()
                        k_back(st)
                        if st[1:3] == (0, 1):
                            wave2()
            qpend = unit_front("q", 0, 0, s12_pool=False)
            while kq:
                k_back(kq.popleft())

            # ksum_flat = sum of the 4 j-chunks
            kv4 = ksum_parts.rearrange("p (t j) -> p t j", j=NJ)
            kst1 = cp.tile([P, FT], F32, tag="kst1")
            kst2 = cp.tile([P, FT], F32, tag="kst2")
            ksum_flat = cp.tile([P, FT], F32, tag="ksum_flat")
            nc.vector.tensor_tensor(kst1[:], kv4[:, :, 0], kv4[:, :, 1], ALU.add)
            nc.vector.tensor_tensor(kst2[:], kv4[:, :, 2], kv4[:, :, 3], ALU.add)
            nc.vector.tensor_tensor(ksum_flat[:], kst1[:], kst2[:], ALU.add)

            # xk in transposed form xkT[p, kt, h] = sum_s x[s, kt*128+p] psk[s, h]
            xkP = pp_s.tile([P, KT * H], F32, tag="small")
            xkv = xkP.rearrange("p (kt h) -> p kt h", h=H)
            for kt in range(KT):
                for st in range(NST):
                    nc.tensor.matmul(
                        xkv[:, kt], xn[:, st, kt * P:(kt + 1) * P],
                        psk_nat[:, st, :],
                        start=(st == 0), stop=(st == NST - 1),
                    )
            xk_sb = cp.tile([P, KT * H], F32, tag="xk_sb")
            nc.vector.tensor_copy(xk_sb[:], xkP[:])
            # psktot = colsum of psk
            ptP = pp_s.tile([1, H], F32, tag="small")
            for st in range(NST):
                nc.tensor.matmul(
                    ptP[:], cpack[:, 2 * P + 2:2 * P + 3], psk_nat[:, st, :],
                    start=(st == 0), stop=(st == NST - 1),
                )
            psktot_f = cp.tile([1, H], F32, tag="psktot_f")
            nc.vector.tensor_copy(psktot_f[:], ptP[:])

            # ============ collective (pairs share a batch) ============
            with nc.allow_non_contiguous_dma(reason="70KB collective bundle"):
                nc.sync.dma_start(
                    cc_i[0:CC_XK].rearrange("(a b) -> a b", a=P), xk_sb[:]
                )
                nc.sync.dma_start(
                    cc_i[CC_XK:CC_XK + CC_KS].rearrange("(a b) -> a b", a=P),
                    ksum_flat[:],
                )
                nc.sync.dma_start(
                    cc_i[CC_XK + CC_KS:CC_LEN].rearrange("(a b) -> a b", a=1),
                    psktot_f[:],
                )
            if collective:
                nc.gpsimd.collective_compute(
                    "AllReduce",
                    ALU.add,
                    replica_groups=[[0, 1], [2, 3], [4, 5], [6, 7]],
                    ins=[cc_i.opt()],
                    outs=[cc_o.opt()],
                )
            else:  # timing-model variant: TimelineSim can't model collectives
                nc.sync.dma_start(cc_o[:], cc_i[:])

            # ============ Q path: phi production (collective-independent) ==
            def q_front(t, j):
                return unit_front("q", t, j, s12_pool=(t % 2 == 1))

            def q_back(state):
                _, t, j, _, _ = state
                unit_back(
                    state, phiq_all[:, t, j * 512:(j + 1) * 512],
                    relu_act=(t % 4 == 2),
                )

            qq = _c.deque([qpend])
            for j in range(0, 2):
                for t in range(FT):
                    if j == 0 and t == 0:
                        continue
                    qq.append(q_front(t, j))
                    if len(qq) > 2:
                        q_back(qq.popleft())

            # ---- collective consumers: emitted ~2 j-chunks (35us) after the
            # all-reduce was issued, so even a slow collective is off the
            # critical path by the time these hit the engine queues ----
            xk_r = cp.tile([P, KT * H], F32, tag="xk_r")
            ksum_r = cp.tile([P, FT], F32, tag="ksum_r")
            psktot_r = cp.tile([1, H], F32, tag="psktot_r")
            with nc.allow_non_contiguous_dma(reason="70KB collective bundle"):
                nc.sync.dma_start(
                    xk_r[:], cc_o[0:CC_XK].rearrange("(a b) -> a b", a=P)
                )
                nc.sync.dma_start(
                    ksum_r[:],
                    cc_o[CC_XK:CC_XK + CC_KS].rearrange("(a b) -> a b", a=P),
                )
                nc.sync.dma_start(
                    psktot_r[:],
                    cc_o[CC_XK + CC_KS:CC_LEN].rearrange("(a b) -> a b", a=1),
                )
            xkT8 = cp.tile([P, KT, H], BF16, tag="xkT8")
            nc.gpsimd.tensor_copy(
                xkT8[:], xk_r.rearrange("p (kt h) -> p kt h", h=H)
            )
            psktot_rb = cp.tile([1, H], BF16, tag="psktot_rb")
            nc.vector.tensor_copy(psktot_rb[:], psktot_r[:])
            # qones[p, m, t]: m 0/1 = head-half ones, m 2/3 = ksum halves
            qones = cp.tile([P, 4, FT], BF16, tag="qones")
            nc.vector.memset(qones[:], 0.0)
            nc.vector.memset(qones[0:64, 0, :], 1.0)
            nc.vector.memset(qones[64:P, 1, :], 1.0)
            nc.gpsimd.tensor_copy(qones[0:64, 2, :], ksum_r[0:64, :])
            nc.gpsimd.tensor_copy(qones[64:P, 3, :], ksum_r[64:P, :])

            # kvsum / Wo2
            kvsum_f = cp.tile([P, FT], F32, tag="kvsum_f")
            for t in range(FT):
                tsl = slice(t * P, (t + 1) * P)
                kvP = pp_s.tile([P, H], F32, tag="small")
                for kt in range(KT):
                    nc.tensor.matmul(
                        kvP[:], wvT[:, kt, tsl], xkT8[:, kt, :],
                        start=(kt == 0), stop=False,
                    )
                nc.tensor.matmul(
                    kvP[:], bvb[:, tsl], psktot_rb[:],
                    start=False, stop=True,
                )
                nc.vector.tensor_copy(
                    kvsum_f[0:64, t:t + 1], kvP[0:64, 2 * t:2 * t + 1]
                )
                nc.vector.tensor_copy(
                    kvsum_f[64:P, t:t + 1], kvP[64:P, 2 * t + 1:2 * t + 2]
                )
            kvsel = cp.tile([P, KT, H], BF16, tag="kvsel")
            nc.vector.memset(kvsel[:], 0.0)
            kvselv = kvsel.rearrange("p kt h -> p (kt h)")
            # element (t, 2t [+1]) of the [8,16] grid = flat index 18t [+1]
            nc.vector.tensor_copy(
                kvselv[0:64, 0:127:18], kvsum_f[0:64, :]
            )
            nc.vector.tensor_copy(
                kvselv[64:P, 1:128:18], kvsum_f[64:P, :]
            )
            for half in range(2):
                hsl = slice(half * 512, (half + 1) * 512)
                w2P = pp_s.tile([H, 512], F32, tag="small")
                for kt in range(KT):
                    nc.tensor.matmul(
                        w2P[:], kvsel[:, kt, :], woT[:, kt, hsl],
                        start=(kt == 0), stop=(kt == KT - 1),
                    )
                nc.scalar.copy(wo2ext[0:H, hsl], w2P[:])

            # ============ qd/z/y tail for one j-chunk ====================
            qd_nat = cp.tile([P, NST * FT * 4], F32, tag="qd_nat")
            qdv = qd_nat.rearrange("p (st t m) -> p st t m", st=NST, t=FT)
            den_cl = cp.tile([P, 256], F32, tag="den_cl")
            zr = cp.tile([P, 256], F32, tag="zr")
            zq_c = cp.tile([P, 256], BF16, tag="zq_c")
            zqv = zq_c.rearrange("p (st t hh) -> p st t hh", st=NST, t=FT)
            zrv = zr.rearrange("p (st t hh) -> p st t hh", st=NST, t=FT)
            dclv = den_cl.rearrange("p (st t hh) -> p st t hh", st=NST, t=FT)

            def tail_qd(j):
                qdP = pp_s.tile([P, FT * 16], F32, tag="small")
                qdPv = qdP.rearrange("p (t s m) -> p t s m", t=FT, s=4)
                for t in range(FT):
                    for sub in range(4):
                        st = 4 * j + sub
                        nc.tensor.matmul(
                            qdPv[:, t, sub, :],
                            phiq_all[:, t, st * P:(st + 1) * P],
                            qones[:, :, t],
                        )
                sts = slice(4 * j, 4 * (j + 1))
                nc.vector.tensor_copy(
                    qdv[:, sts, :, :],
                    qdP.rearrange("p (t s m) -> p s t m", t=FT, s=4),
                )

            def tail_z(j):
                sts = slice(4 * j, 4 * (j + 1))
                zsl = slice(64 * j, 64 * (j + 1))
                sts = slice(4 * j, 4 * (j + 1))
                nc.vector.tensor_scalar_max(
                    dclv[:, sts], qdv[:, sts, :, 2:4], EPS
                )
                nc.vector.reciprocal(zr[:, zsl], den_cl[:, zsl])
                nc.vector.tensor_tensor(
                    zqv[:, sts], zrv[:, sts], qdv[:, sts, :, 0:2], ALU.mult
                )
                for sub in range(4):
                    st = 4 * j + sub
                    ssl = slice(st * P, (st + 1) * P)
                    zP = pp_s.tile([H, P], BF16, tag="small")
                    nc.tensor.transpose(
                        zP[:], zq_c[:, st * H:(st + 1) * H], cpack[:, 0:P]
                    )
                    nc.scalar.copy(zqext[0:H, ssl], zP[:])

            def tail_y(j):
                for sub in range(4):
                    st = 4 * j + sub
                    ssl = slice(st * P, (st + 1) * P)
                    ysb = yp.tile([P, DM], F32, tag="ysb")
                    for half in range(2):
                        hsl = slice(half * 512, (half + 1) * 512)
                        yP = pp_s.tile([P, 512], F32, tag="small")
                        nc.tensor.matmul(yP[:], zqext[:, ssl], wo2ext[:, hsl])
                        if half == 0:
                            nc.vector.tensor_copy(ysb[:, hsl], yP[:])
                        else:
                            nc.scalar.copy(ysb[:, hsl], yP[:])
                    nc.sync.dma_start(y_out[ssl, :], ysb[:])

            def tail(j):
                tail_qd(j)
                tail_z(j)
                tail_y(j)

            tail(0)
            for t in range(FT):
                qq.append(q_front(t, 2))
                q_back(qq.popleft())
            tail(1)
            for t in range(0, 4):
                qq.append(q_front(t, 3))
                q_back(qq.popleft())
            tail_qd(2)
            for t in range(4, FT):
                qq.append(q_front(t, 3))
                q_back(qq.popleft())
            while qq:
                q_back(qq.popleft())
            tail_z(2)
            tail_qd(3)
            tail_y(2)
            tail_z(3)
            tail_y(3)

    nc._dbg = dict(
        psk_nat=psk_nat, ksum_parts=ksum_parts, ksum_flat=ksum_flat,
        xk_sb=xk_sb, psktot_f=psktot_f, xkT8=xkT8, ksum_r=ksum_r,
        kvsum_f=kvsum_f, kvsel=kvsel, wo2ext=wo2ext, qones=qones,
        qd_nat=qd_nat, zq_c=zq_c, phiq_all=phiq_all, zqext=zqext,
    )
    nc.finalize()
    return nc


def _consts():
    psign = np.zeros((P, P), np.float32)
    for h in range(2):
        for i in range(32):
            psign[h * 64 + 32 + i, h * 64 + i] = -1.0   # even' = .. - s*odd
            psign[h * 64 + i, h * 64 + 32 + i] = 1.0    # odd'  = .. + s*even
    ones3 = np.zeros((P, 3), np.float32)
    ones3[0:64, 0] = 1.0
    ones3[64:P, 1] = 1.0
    ones3[:, 2] = 1.0
    cpack = np.concatenate(
        [np.eye(P, dtype=np.float32), psign, ones3], axis=1
    ).astype(bf)
    return {"cpack": np.ascontiguousarray(cpack)}


def _perm_rows(w):
    """Per-head [even|odd] row permutation: dest row h*64+pr*32+i comes from
    source row h*64+2i+pr."""
    return np.ascontiguousarray(
        w.reshape(H, 32, 2, -1).transpose(0, 2, 1, 3).reshape(DM, -1)
    )


@functools.lru_cache(maxsize=1)
def _program():
    return build_program()


def make_in_maps(inputs):
    consts = _consts()
    f32 = np.float32

    def wT_tmajor(w):  # [128, t, kt, 128] from permuted W
        a = np.ascontiguousarray(_perm_rows(np.asarray(w, f32)).T.astype(pdt))
        return np.ascontiguousarray(
            a.reshape(KT, P, FT, P).transpose(1, 2, 0, 3)
        )

    def wT_plain(w):   # [128, kt, 1024] = W.T tiles
        a = np.ascontiguousarray(np.asarray(w, f32).T.astype(bf))
        return np.ascontiguousarray(a.reshape(KT, P, DM).transpose(1, 0, 2))

    def bT_perm(b):    # [128, t] f32 permuted bias columns
        bp = np.asarray(b, f32).reshape(H, 32, 2).transpose(0, 2, 1).reshape(DM)
        return np.ascontiguousarray(bp.reshape(FT, P).T)

    bkq = np.concatenate(
        [bT_perm(inputs["bk"]), bT_perm(inputs["bq"])], axis=1
    )
    shared = {
        "wqT": wT_tmajor(inputs["Wq"]),
        "wkT": wT_tmajor(inputs["Wk"]),
        "wvT": wT_plain(inputs["Wv"]),
        "woT": wT_plain(inputs["Wo"]),
        "bkq": np.ascontiguousarray(bkq),
        "bvb": np.asarray(inputs["bv"], f32).astype(bf).reshape(1, DM),
        "bob": np.asarray(inputs["bo"], f32).astype(bf).reshape(1, DM),
        **consts,
    }
    x = np.asarray(inputs["x"], f32)
    pos = np.asarray(inputs["rotary_pos_enc"], f32)
    mask = np.asarray(inputs["padding_mask"], np.int32)
    # cos/sin broadcast tables per seq half: trig[:,0,:]=cos, [:,1,:]=sin
    trig = []
    for hf in range(2):
        sl = slice(hf * SC, (hf + 1) * SC)
        tt = np.zeros((P, 2, SC), bf)
        tt[:, 0, :] = np.tile(pos[sl, 0, 32:64].T.astype(bf), (4, 1))
        tt[:, 1, :] = np.tile(pos[sl, 0, 0:32].T.astype(bf), (4, 1))
        trig.append(np.ascontiguousarray(tt))
    in_maps = []
    for c in range(N_CORES):
        b_, hf = c // 2, c % 2
        sl = slice(hf * SC, (hf + 1) * SC)
        xc = np.ascontiguousarray(x[b_, sl])                     # [2048,1024]
        xTc = np.ascontiguousarray(
            xc.T.astype(pdt).reshape(KT, P, 2, 1024).transpose(1, 2, 0, 3)
        )
        xnc = np.ascontiguousarray(
            xc.astype(bf).reshape(NST, P, DM).transpose(1, 0, 2)
        )
        mnegc = np.ones((1, SC + P), np.float32)
        mnegc[0, 0:SC] = mask[b_, sl].astype(f32) * -1e4
        in_maps.append(
            {
                "xT": xTc,
                "xn": xnc,
                "trig": trig[hf],
                "mnegc": np.ascontiguousarray(mnegc.astype(bf)),
                **shared,
            }
        )
    return in_maps


def run(inputs, **kwargs):
    nc = _program()
    in_maps = make_in_maps(inputs)
    res = run_bass_kernel_spmd(
        nc, in_maps, core_ids=list(range(N_CORES)), **kwargs
    )
    out = np.zeros((B, S, DM), np.float32)
    for c in range(N_CORES):
        b_, hf = c // 2, c % 2
        out[b_, hf * SC:(hf + 1) * SC, :] = res.results[c]["y"]
    return out, res


def kernel(**inputs) -> np.ndarray:
    out, _ = run(inputs)
    return out


# revision 80
# speedup vs baseline: 1.0661x; 1.0147x over previous
"""Trainium2 Bass kernel for nn_MultiHeadAttention_27711128994021.

Reference math (faithful to the oracle, including its independent-sum einsum):
  q = x@Wq.T+bq ; k = x@Wk.T+bk ; v = x@Wv.T+bv       (B,S,H,D)
  rq, rk = rope(pos, q, k)
  phi_q = elu(rq)+1 ; phi_k = (elu(rk)+1) * notpad
  attn[b,s,h,v] = z[b,h,s] * (sum_q phi_q[b,s,h,q]) * (sum_k kv[b,h,v,k])
    with kv = einsum("bshv,bshk->bhvk", v, phi_k), z = 1/clip(phi_q . k_sum)
  out = attn @ Wo.T + bo

Because q and k are summed independently in the attn einsum, attention is
rank-1 per (b,h):  attn = zq[s,h] * kvsum[h,v]  where
  kvsum[h,v] = sum_s v[s,hv] * psk[s,h],  psk = rowsum(phi_k)
so the V projection collapses:  kvsum = (Wv @ (psk.T @ x).T)_head-diag  and
the out projection collapses to rank-16:  y = zq @ Wo2 + bo with
  Wo2[h,n] = sum_v kvsum[h,v] Wo[n,64h+v].
Only the Q and K projections remain as large matmuls.

Sharding: 8 cores = (batch b, seq half). Per core: 2048 rows of one batch.
Cross-core data: all-reduce (pairs) of xk=psk.T@x [16,1024], k_sum [1024],
psktot [16] — 70KB, hidden behind the Q-path phi production.

All operand preparation (bf16 casts, transposes, per-head [even|odd] row
permutation of Wq/Wk, cos/sin broadcast tables, permuted bias columns) is
done host-side in numpy; the device only streams compute.

Padding mask is applied ADDITIVELY: a rank-1 matmul adds -1e4 to every
masked key column inside the rope PSUM accumulation, so
phi_k = min(exp(v),1)+relu(v) (identical formula to phi_q) comes out zero
for masked rows. elu(x)+1 == min(exp(x),1) + max(x,0) exactly.

phi_q is stashed fully in SBUF so its production has no dependency on the
collective; the qd/z/y tail consumes it j-chunk by j-chunk afterwards,
giving the all-reduce ~35us of slack off the critical path.

Schedule: Q/K projections run as fp8e4m3 DoubleRow matmuls (4 chained
k-pair passes); each (t, j) unit is software-pipelined two deep
(proj+evac+s12 fronts run ahead of rope+exp+phi backs), with rope
accumulating into the *same* PSUM slot the projection used, so one
1-bank PSUM slot per unit gives six units in flight. The Q path uses
the exact identity phi = min(exp(v), relu(v)+1) (one tensor_scalar +
one 2x-mode tensor_tensor); the K path keeps min(exp,1)+relu so the
ksum accumulation can ride the scalar_tensor_tensor. Work is spread
across engines: ACT does evac/exp (+half the K relus), DVE the rest,
and the Pool engine takes half the Q (and a quarter of the K) s12
products plus post-collective copies (Pool cannot run TensorScalarPtr
or touch PSUM on real HW). Weight/x loads stream in two waves
(SP/HWDGE for big K operands, Pool/SWDGE for tables), with wave 2
gated behind an early K product to keep the shared DMA engines clear
during ramp.
"""

import functools

import numpy as np
import ml_dtypes

import concourse.bass as bass
import concourse.mybir as mybir
import concourse.tile as tile
from concourse import bacc
from concourse.bass_utils import run_bass_kernel_spmd

F32 = mybir.dt.float32
BF16 = mybir.dt.bfloat16
FP8 = mybir.dt.float8e4
I32 = mybir.dt.int32
PROJ_FP8 = True   # fp8e4m3 + DoubleRow for the Q/K projections
PDT = FP8 if PROJ_FP8 else BF16
AF = mybir.ActivationFunctionType
ALU = mybir.AluOpType

P = 128
B, S, H, D = 4, 4096, 16, 64
DM = H * D            # 1024
SC = 2048             # seq rows per core
KT = DM // P          # 8 k tiles
FT = DM // P          # 8 feature tiles (2 heads each)
NJ = SC // 512        # 4 s-chunks of 512
NST = SC // P         # 16 seq tiles of 128
EPS = 1e-6
N_CORES = 8
CC_XK, CC_KS, CC_PT = P * KT * H, P * FT, 16   # collective bundle sections
CC_LEN = CC_XK + CC_KS + CC_PT

bf = ml_dtypes.bfloat16
f8 = ml_dtypes.float8_e4m3fn
pdt = f8 if PROJ_FP8 else bf


def build_program(collective=True):
    nc = bacc.Bacc(
        "TRN2", target_bir_lowering=False, debug=False, num_devices=N_CORES
    )

    # ---- I/O (everything already laid out / cast host-side) ----
    xT_in = nc.dram_tensor("xT", [P, 2, KT, 1024], PDT, kind="ExternalInput").ap()
    xn_in = nc.dram_tensor("xn", [P, NST, DM], BF16, kind="ExternalInput").ap()
    wqT_in = nc.dram_tensor("wqT", [P, FT, KT, P], PDT, kind="ExternalInput").ap()
    wkT_in = nc.dram_tensor("wkT", [P, FT, KT, P], PDT, kind="ExternalInput").ap()
    wvT_in = nc.dram_tensor("wvT", [P, KT, DM], BF16, kind="ExternalInput").ap()
    woT_in = nc.dram_tensor("woT", [P, KT, DM], BF16, kind="ExternalInput").ap()
    # trig[:,0,:]=cos, [:,1,:]=sin ; cpack = [ident|psign|ones3] ;
    # mnegc = [-1e4*mask | ones-row] ; bkq = [bkT|bqT]
    trig_in = nc.dram_tensor("trig", [P, 2, SC], BF16, kind="ExternalInput").ap()
    cpack_in = nc.dram_tensor("cpack", [P, 259], BF16, kind="ExternalInput").ap()
    mnegc_in = nc.dram_tensor("mnegc", [1, SC + P], BF16, kind="ExternalInput").ap()
    bkq_in = nc.dram_tensor("bkq", [P, 2 * FT], F32, kind="ExternalInput").ap()
    bvb_in = nc.dram_tensor("bvb", [1, DM], BF16, kind="ExternalInput").ap()
    bob_in = nc.dram_tensor("bob", [1, DM], BF16, kind="ExternalInput").ap()
    y_out = nc.dram_tensor("y", [SC, DM], F32, kind="ExternalOutput").ap()

    with tile.TileContext(nc) as tc:
        with (
            tc.tile_pool(name="const", bufs=1) as cp,
            tc.tile_pool(name="work", bufs=5 if PROJ_FP8 else 2) as wp,
            tc.tile_pool(name="ypool", bufs=3) as yp,
            tc.tile_pool(name="pbig", bufs=6, space="PSUM") as pb,
            tc.tile_pool(name="psmall", bufs=2, space="PSUM") as pp_s,
            tc.tile_pool(name="dram", bufs=1, space="DRAM") as dp,
        ):
            cc_i = dp.tile([CC_LEN], F32, tag="cc_i")
            cc_o = dp.tile([CC_LEN], F32, tag="cc_o")

            # ---- SBUF residents ----
            xT = cp.tile([P, 2, KT, 1024], PDT, tag="xT")
            xn = cp.tile([P, NST, DM], BF16, tag="xn")
            wqT = cp.tile([P, FT, KT, P], PDT, tag="wqT")
            wkT = cp.tile([P, FT, KT, P], PDT, tag="wkT")
            wvT = cp.tile([P, KT, DM], BF16, tag="wvT")
            woT = cp.tile([P, KT, DM], BF16, tag="woT")
            trig = cp.tile([P, 2, SC], BF16, tag="trig")
            cpack = cp.tile([P, 259], BF16, tag="cpack")
            mnegc = cp.tile([1, SC + P], BF16, tag="mnegc")
            bkq = cp.tile([P, 2 * FT], F32, tag="bkq")
            bkT = bkq[:, 0:FT]
            bqT = bkq[:, FT:2 * FT]
            bvb = cp.tile([1, DM], BF16, tag="bvb")
            phiq_all = cp.tile([P, FT, SC], BF16, tag="phiq_all")

            # ---- wave 1 loads: big K-path operands on the SP/HWDGE queue,
            # small tables on the Pool/SWDGE queue (parallel issue paths) ----
            nc.sync.dma_start(wkT[:, 0:1], wkT_in[:, 0:1])
            nc.sync.dma_start(xT[:, 0, 0:2], xT_in[:, 0, 0:2])
            nc.sync.dma_start(xT[:, 0, 2:4], xT_in[:, 0, 2:4])
            nc.sync.dma_start(wkT[:, 1:2], wkT_in[:, 1:2])
            nc.sync.dma_start(xT[:, 0, 4:KT], xT_in[:, 0, 4:KT])
            nc.sync.dma_start(wkT[:, 2:4], wkT_in[:, 2:4])
            nc.sync.dma_start(wkT[:, 4:FT], wkT_in[:, 4:FT])
            nc.sync.dma_start(xT[:, 1, 0:4], xT_in[:, 1, 0:4])
            nc.sync.dma_start(xT[:, 1, 4:KT], xT_in[:, 1, 4:KT])
            nc.gpsimd.dma_start(trig[:, :, 0:1024], trig_in[:, :, 0:1024])
            nc.gpsimd.dma_start(bkq[:], bkq_in)
            nc.gpsimd.dma_start(cpack[:], cpack_in)
            nc.gpsimd.dma_start(mnegc[:], mnegc_in)
            nc.gpsimd.dma_start(trig[:, :, 1024:SC], trig_in[:, :, 1024:SC])
            # preload the Exp activation table while ACT is otherwise idle
            actwarm = cp.tile([1, 2], BF16, tag="actwarm")
            nc.scalar.activation(actwarm[:], trig[0:1, 0, 0:2], AF.Exp)

            # accumulators
            ksum_parts = cp.tile([P, FT * NJ], F32, tag="ksum_parts")
            psk_nat = cp.tile([P, NST, H], BF16, tag="psk_nat")
            zqext = cp.tile([H + 1, SC], BF16, tag="zqext")
            nc.vector.memset(zqext[:], 1.0)
            wo2ext = cp.tile([H + 1, DM], BF16, tag="wo2ext")
            nc.gpsimd.dma_start(wo2ext[H:H + 1, :], bob_in)

            # ---- wave 2 loads (Pool/SWDGE queue), emitted mid-K-path.
            # The scheduler reorders independent queue entries, so gate each
            # transfer behind an early K-path product with a real WAW dep
            # (corner-copy into the dst).
            def wave2():
                def gated(corner, dst, src):
                    nc.gpsimd.tensor_copy(corner, psk_nat[0:2, 4, 0:2])
                    nc.gpsimd.dma_start(dst, src)

                gated(wqT[0:2, 0, 0, 0:2], wqT[:], wqT_in)
                for sg in range(4):
                    gated(
                        xn[0:2, 4 * sg, 0:2],
                        xn[:, 4 * sg:4 * sg + 4], xn_in[:, 4 * sg:4 * sg + 4]
                    )
                gated(wvT[0:2, 0, 0:2], wvT[:], wvT_in)
                gated(woT[0:2, 0, 0:2], woT[:], woT_in)
                nc.gpsimd.dma_start(bvb[:], bvb_in)

            def unit_front(path, t, j, s12_pool=False):
                """proj chain + bias evac + cos/sin products, 512-wide."""
                jsl = slice(j * 512, (j + 1) * 512)
                J, jh = j // 2, (j % 2) * 512
                wT = wkT if path == "k" else wqT
                bofs = 0 if path == "k" else FT
                projP = pb.tile([P, 512], F32, tag="pb")
                if PROJ_FP8:
                    for kp in range(KT // 2):
                        nc.tensor.matmul(
                            projP[:], wT[:, t, 2 * kp:2 * kp + 2],
                            xT[:, J, 2 * kp:2 * kp + 2, jh:jh + 512],
                            start=(kp == 0), stop=(kp == KT // 2 - 1),
                            perf_mode=mybir.MatmulPerfMode.DoubleRow,
                        )
                else:
                    for kt in range(KT):
                        nc.tensor.matmul(
                            projP[:], wT[:, t, kt], xT[:, J, kt, jh:jh + 512],
                            start=(kt == 0), stop=(kt == KT - 1),
                        )
                ck = wp.tile([P, 1, 512], BF16, tag="ck")
                nc.scalar.activation(
                    ck[:, 0], projP[:], AF.Identity,
                    bias=bkq[:, bofs + t:bofs + t + 1],
                )
                # s12[:,0,:] = ck*cos, s12[:,1,:] = ck*sin in one op via a
                # stride-0 broadcast of ck over the middle dim
                s12 = wp.tile([P, 2, 512], BF16, tag="s12")
                eng = nc.gpsimd if s12_pool else nc.vector
                eng.tensor_tensor(
                    s12[:], ck[:].broadcast_to([P, 2, 512]), trig[:, :, jsl],
                    ALU.mult,
                )
                return (path, t, j, projP, s12)

            def unit_back(state, out_phi, stt_pool=False, relu_act=True):
                """rope (reusing the proj PSUM slot) + feature map."""
                path, t, j, projP, s12 = state
                nc.tensor.matmul(
                    projP[:], cpack[:, 0:P], s12[:, 0], start=True, stop=False
                )
                nc.tensor.matmul(
                    projP[:], cpack[:, P:2 * P], s12[:, 1],
                    start=False, stop=(path == "q"),
                )
                if path == "k":
                    nc.tensor.matmul(
                        projP[:], mnegc[0:1, SC:SC + P],
                        mnegc[0:1, j * 512:(j + 1) * 512],
                        start=False, stop=True,
                    )
                e = wp.tile([P, 512], BF16, tag="e")
                nc.scalar.activation(e[:], projP[:], AF.Exp)
                r = wp.tile([P, 512], BF16, tag="r")
                if path == "k":
                    # phi = min(e,1) + r with the ksum accumulation riding on
                    # the scalar_tensor_tensor
                    if relu_act:
                        nc.scalar.activation(r[:], projP[:], AF.Relu)
                    else:
                        nc.vector.tensor_scalar_max(r[:], projP[:], 0.0)
                    nc.vector.scalar_tensor_tensor(
                        out_phi, e[:], 1.0, r[:], ALU.min, ALU.add,
                        accum_out=ksum_parts[:, t * NJ + j:t * NJ + j + 1],
                    )
                else:
                    # phi = min(exp(v), relu(v)+1) exactly; the min runs in
                    # the DVE 2x bf16 mode
                    nc.vector.tensor_scalar(
                        r[:], projP[:], 0.0, 1.0, ALU.max, ALU.add
                    )
                    nc.vector.tensor_tensor(out_phi, e[:], r[:], ALU.min)

            # ============ K path: software-pipelined, j-outer ============
            def psk_stage(phik, t, j):
                pskT = pp_s.tile([P, 8], F32, tag="small")
                for sub in range(4):
                    nc.tensor.matmul(
                        pskT[:, 2 * sub:2 * sub + 2],
                        phik[:, sub * P:(sub + 1) * P],
                        cpack[:, 2 * P:2 * P + 2],
                    )
                nc.vector.tensor_copy(
                    psk_nat[:, 4 * j:4 * j + 4, 2 * t:2 * t + 2],
                    pskT.rearrange("p (st hh) -> p st hh", hh=2),
                )

            def k_back(state):
                path, t, j, _, _ = state
                phik = wp.tile([P, 512], BF16, tag="phik")
                unit_back(state, phik[:], relu_act=(t % 2 == 0))
                psk_stage(phik, t, j)

            import collections as _c
            kq = _c.deque()
            for j in range(NJ):
                for t in range(FT):
                    kq.append(unit_front("k", t, j, s12_pool=(t % 4 == 3 and j > 0)))
                    if len(kq) > 2:
                        st = kq.popleft()
                        k_back(st)
                        if st[1:3] == (0, 1):
                            wave2()
            qpend = unit_front("q", 0, 0, s12_pool=False)
            while kq:
                k_back(kq.popleft())

            # ksum_flat = sum of the 4 j-chunks
            kv4 = ksum_parts.rearrange("p (t j) -> p t j", j=NJ)
            kst1 = cp.tile([P, FT], F32, tag="kst1")
            kst2 = cp.tile([P, FT], F32, tag="kst2")
            ksum_flat = cp.tile([P, FT], F32, tag="ksum_flat")
            nc.vector.tensor_tensor(kst1[:], kv4[:, :, 0], kv4[:, :, 1], ALU.add)
            nc.vector.tensor_tensor(kst2[:], kv4[:, :, 2], kv4[:, :, 3], ALU.add)
            nc.vector.tensor_tensor(ksum_flat[:], kst1[:], kst2[:], ALU.add)

            # xk in transposed form xkT[p, kt, h] = sum_s x[s, kt*128+p] psk[s, h]
            xkP = pp_s.tile([P, KT * H], F32, tag="small")
            xkv = xkP.rearrange("p (kt h) -> p kt h", h=H)
            for kt in range(KT):
                for st in range(NST):
                    nc.tensor.matmul(
                        xkv[:, kt], xn[:, st, kt * P:(kt + 1) * P],
                        psk_nat[:, st, :],
                        start=(st == 0), stop=(st == NST - 1),
                    )
            xk_sb = cp.tile([P, KT * H], F32, tag="xk_sb")
            nc.vector.tensor_copy(xk_sb[:], xkP[:])
            # psktot = colsum of psk
            ptP = pp_s.tile([1, H], F32, tag="small")
            for st in range(NST):
                nc.tensor.matmul(
                    ptP[:], cpack[:, 2 * P + 2:2 * P + 3], psk_nat[:, st, :],
                    start=(st == 0), stop=(st == NST - 1),
                )
            psktot_f = cp.tile([1, H], F32, tag="psktot_f")
            nc.vector.tensor_copy(psktot_f[:], ptP[:])

            # ============ collective (pairs share a batch) ============
            with nc.allow_non_contiguous_dma(reason="70KB collective bundle"):
                nc.sync.dma_start(
                    cc_i[0:CC_XK].rearrange("(a b) -> a b", a=P), xk_sb[:]
                )
                nc.sync.dma_start(
                    cc_i[CC_XK:CC_XK + CC_KS].rearrange("(a b) -> a b", a=P),
                    ksum_flat[:],
                )
                nc.sync.dma_start(
                    cc_i[CC_XK + CC_KS:CC_LEN].rearrange("(a b) -> a b", a=1),
                    psktot_f[:],
                )
            if collective:
                nc.gpsimd.collective_compute(
                    "AllReduce",
                    ALU.add,
                    replica_groups=[[0, 1], [2, 3], [4, 5], [6, 7]],
                    ins=[cc_i.opt()],
                    outs=[cc_o.opt()],
                )
            else:  # timing-model variant: TimelineSim can't model collectives
                nc.sync.dma_start(cc_o[:], cc_i[:])

            # ============ Q path: phi production (collective-independent) ==
            def q_front(t, j):
                return unit_front("q", t, j, s12_pool=(t % 2 == 1))

            def q_back(state):
                _, t, j, _, _ = state
                unit_back(
                    state, phiq_all[:, t, j * 512:(j + 1) * 512],
                    relu_act=(t % 4 == 2),
                )

            qq = _c.deque([qpend])
            for j in range(0, 2):
                for t in range(FT):
                    if j == 0 and t == 0:
                        continue
                    qq.append(q_front(t, j))
                    if len(qq) > 2:
                        q_back(qq.popleft())

            # ---- collective consumers: emitted ~2 j-chunks (35us) after the
            # all-reduce was issued, so even a slow collective is off the
            # critical path by the time these hit the engine queues ----
            xk_r = cp.tile([P, KT * H], F32, tag="xk_r")
            ksum_r = cp.tile([P, FT], F32, tag="ksum_r")
            psktot_r = cp.tile([1, H], F32, tag="psktot_r")
            with nc.allow_non_contiguous_dma(reason="70KB collective bundle"):
                nc.sync.dma_start(
                    xk_r[:], cc_o[0:CC_XK].rearrange("(a b) -> a b", a=P)
                )
                nc.sync.dma_start(
                    ksum_r[:],
                    cc_o[CC_XK:CC_XK + CC_KS].rearrange("(a b) -> a b", a=P),
                )
                nc.sync.dma_start(
                    psktot_r[:],
                    cc_o[CC_XK + CC_KS:CC_LEN].rearrange("(a b) -> a b", a=1),
                )
            xkT8 = cp.tile([P, KT, H], BF16, tag="xkT8")
            nc.gpsimd.tensor_copy(
                xkT8[:], xk_r.rearrange("p (kt h) -> p kt h", h=H)
            )
            psktot_rb = cp.tile([1, H], BF16, tag="psktot_rb")
            nc.vector.tensor_copy(psktot_rb[:], psktot_r[:])
            # qones[p, m, t]: m 0/1 = head-half ones, m 2/3 = ksum halves
            qones = cp.tile([P, 4, FT], BF16, tag="qones")
            nc.vector.memset(qones[:], 0.0)
            nc.vector.memset(qones[0:64, 0, :], 1.0)
            nc.vector.memset(qones[64:P, 1, :], 1.0)
            nc.gpsimd.tensor_copy(qones[0:64, 2, :], ksum_r[0:64, :])
            nc.gpsimd.tensor_copy(qones[64:P, 3, :], ksum_r[64:P, :])

            # kvsum / Wo2
            kvsum_f = cp.tile([P, FT], F32, tag="kvsum_f")
            for t in range(FT):
                tsl = slice(t * P, (t + 1) * P)
                kvP = pp_s.tile([P, H], F32, tag="small")
                for kt in range(KT):
                    nc.tensor.matmul(
                        kvP[:], wvT[:, kt, tsl], xkT8[:, kt, :],
                        start=(kt == 0), stop=False,
                    )
                nc.tensor.matmul(
                    kvP[:], bvb[:, tsl], psktot_rb[:],
                    start=False, stop=True,
                )
                nc.vector.tensor_copy(
                    kvsum_f[0:64, t:t + 1], kvP[0:64, 2 * t:2 * t + 1]
                )
                nc.vector.tensor_copy(
                    kvsum_f[64:P, t:t + 1], kvP[64:P, 2 * t + 1:2 * t + 2]
                )
            kvsel = cp.tile([P, KT, H], BF16, tag="kvsel")
            nc.vector.memset(kvsel[:], 0.0)
            kvselv = kvsel.rearrange("p kt h -> p (kt h)")
            # element (t, 2t [+1]) of the [8,16] grid = flat index 18t [+1]
            nc.vector.tensor_copy(
                kvselv[0:64, 0:127:18], kvsum_f[0:64, :]
            )
            nc.vector.tensor_copy(
                kvselv[64:P, 1:128:18], kvsum_f[64:P, :]
            )
            for half in range(2):
                hsl = slice(half * 512, (half + 1) * 512)
                w2P = pp_s.tile([H, 512], F32, tag="small")
                for kt in range(KT):
                    nc.tensor.matmul(
                        w2P[:], kvsel[:, kt, :], woT[:, kt, hsl],
                        start=(kt == 0), stop=(kt == KT - 1),
                    )
                nc.scalar.copy(wo2ext[0:H, hsl], w2P[:])

            # ============ qd/z/y tail for one j-chunk ====================
            qd_nat = cp.tile([P, NST * FT * 4], F32, tag="qd_nat")
            qdv = qd_nat.rearrange("p (st t m) -> p st t m", st=NST, t=FT)
            den_cl = cp.tile([P, 256], F32, tag="den_cl")
            zr = cp.tile([P, 256], F32, tag="zr")
            zq_c = cp.tile([P, 256], BF16, tag="zq_c")
            zqv = zq_c.rearrange("p (st t hh) -> p st t hh", st=NST, t=FT)
            zrv = zr.rearrange("p (st t hh) -> p st t hh", st=NST, t=FT)
            dclv = den_cl.rearrange("p (st t hh) -> p st t hh", st=NST, t=FT)

            def tail_qd(j):
                qdP = pp_s.tile([P, FT * 16], F32, tag="small")
                qdPv = qdP.rearrange("p (t s m) -> p t s m", t=FT, s=4)
                for t in range(FT):
                    for sub in range(4):
                        st = 4 * j + sub
                        nc.tensor.matmul(
                            qdPv[:, t, sub, :],
                            phiq_all[:, t, st * P:(st + 1) * P],
                            qones[:, :, t],
                        )
                sts = slice(4 * j, 4 * (j + 1))
                nc.vector.tensor_copy(
                    qdv[:, sts, :, :],
                    qdP.rearrange("p (t s m) -> p s t m", t=FT, s=4),
                )

            def tail_z(j):
                sts = slice(4 * j, 4 * (j + 1))
                zsl = slice(64 * j, 64 * (j + 1))
                sts = slice(4 * j, 4 * (j + 1))
                nc.vector.tensor_scalar_max(
                    dclv[:, sts], qdv[:, sts, :, 2:4], EPS
                )
                nc.vector.reciprocal(zr[:, zsl], den_cl[:, zsl])
                nc.vector.tensor_tensor(
                    zqv[:, sts], zrv[:, sts], qdv[:, sts, :, 0:2], ALU.mult
                )
                for sub in range(4):
                    st = 4 * j + sub
                    ssl = slice(st * P, (st + 1) * P)
                    zP = pp_s.tile([H, P], BF16, tag="small")
                    nc.tensor.transpose(
                        zP[:], zq_c[:, st * H:(st + 1) * H], cpack[:, 0:P]
                    )
                    nc.scalar.copy(zqext[0:H, ssl], zP[:])

            def tail_y(j):
                for sub in range(4):
                    st = 4 * j + sub
                    ssl = slice(st * P, (st + 1) * P)
                    ysb = yp.tile([P, DM], F32, tag="ysb")
                    for half in range(2):
                        hsl = slice(half * 512, (half + 1) * 512)
                        yP = pp_s.tile([P, 512], F32, tag="small")
                        nc.tensor.matmul(yP[:], zqext[:, ssl], wo2ext[:, hsl])
                        if half == 0:
                            nc.vector.tensor_copy(ysb[:, hsl], yP[:])
                        else:
                            nc.scalar.copy(ysb[:, hsl], yP[:])
                    nc.sync.dma_start(y_out[ssl, :], ysb[:])

            def tail(j):
                tail_qd(j)
                tail_z(j)
                tail_y(j)

            def qd_one(t, j):
                qdT = pp_s.tile([P, 16], F32, tag="small")
                for sub in range(4):
                    st = 4 * j + sub
                    nc.tensor.matmul(
                        qdT[:, 4 * sub:4 * sub + 4],
                        phiq_all[:, t, st * P:(st + 1) * P],
                        qones[:, :, t],
                    )
                nc.vector.tensor_copy(
                    qdv[:, 4 * j:4 * j + 4, t, :],
                    qdT.rearrange("p (s m) -> p s m", m=4),
                )

            tail(0)
            for t in range(FT):
                qq.append(q_front(t, 2))
                q_back(qq.popleft())
            tail(1)
            for t in range(0, 3):
                qq.append(q_front(t, 3))
                st_done = qq.popleft()
                q_back(st_done)
                if st_done[2] == 3:
                    qd_one(st_done[1], 3)
            tail_qd(2)
            for t in range(3, 6):
                qq.append(q_front(t, 3))
                st_done = qq.popleft()
                q_back(st_done)
                if st_done[2] == 3:
                    qd_one(st_done[1], 3)
            tail_z(2)
            tail_y(2)
            for t in range(6, FT):
                qq.append(q_front(t, 3))
                st_done = qq.popleft()
                q_back(st_done)
                if st_done[2] == 3:
                    qd_one(st_done[1], 3)
            while qq:
                st_done = qq.popleft()
                q_back(st_done)
                if st_done[2] == 3:
                    qd_one(st_done[1], 3)
            tail_z(3)
            tail_y(3)

    nc._dbg = dict(
        psk_nat=psk_nat, ksum_parts=ksum_parts, ksum_flat=ksum_flat,
        xk_sb=xk_sb, psktot_f=psktot_f, xkT8=xkT8, ksum_r=ksum_r,
        kvsum_f=kvsum_f, kvsel=kvsel, wo2ext=wo2ext, qones=qones,
        qd_nat=qd_nat, zq_c=zq_c, phiq_all=phiq_all, zqext=zqext,
    )
    nc.finalize()
    return nc


def _consts():
    psign = np.zeros((P, P), np.float32)
    for h in range(2):
        for i in range(32):
            psign[h * 64 + 32 + i, h * 64 + i] = -1.0   # even' = .. - s*odd
            psign[h * 64 + i, h * 64 + 32 + i] = 1.0    # odd'  = .. + s*even
    ones3 = np.zeros((P, 3), np.float32)
    ones3[0:64, 0] = 1.0
    ones3[64:P, 1] = 1.0
    ones3[:, 2] = 1.0
    cpack = np.concatenate(
        [np.eye(P, dtype=np.float32), psign, ones3], axis=1
    ).astype(bf)
    return {"cpack": np.ascontiguousarray(cpack)}


def _perm_rows(w):
    """Per-head [even|odd] row permutation: dest row h*64+pr*32+i comes from
    source row h*64+2i+pr."""
    return np.ascontiguousarray(
        w.reshape(H, 32, 2, -1).transpose(0, 2, 1, 3).reshape(DM, -1)
    )


@functools.lru_cache(maxsize=1)
def _program():
    return build_program()


def make_in_maps(inputs):
    consts = _consts()
    f32 = np.float32

    def wT_tmajor(w):  # [128, t, kt, 128] from permuted W
        a = np.ascontiguousarray(_perm_rows(np.asarray(w, f32)).T.astype(pdt))
        return np.ascontiguousarray(
            a.reshape(KT, P, FT, P).transpose(1, 2, 0, 3)
        )

    def wT_plain(w):   # [128, kt, 1024] = W.T tiles
        a = np.ascontiguousarray(np.asarray(w, f32).T.astype(bf))
        return np.ascontiguousarray(a.reshape(KT, P, DM).transpose(1, 0, 2))

    def bT_perm(b):    # [128, t] f32 permuted bias columns
        bp = np.asarray(b, f32).reshape(H, 32, 2).transpose(0, 2, 1).reshape(DM)
        return np.ascontiguousarray(bp.reshape(FT, P).T)

    bkq = np.concatenate(
        [bT_perm(inputs["bk"]), bT_perm(inputs["bq"])], axis=1
    )
    shared = {
        "wqT": wT_tmajor(inputs["Wq"]),
        "wkT": wT_tmajor(inputs["Wk"]),
        "wvT": wT_plain(inputs["Wv"]),
        "woT": wT_plain(inputs["Wo"]),
        "bkq": np.ascontiguousarray(bkq),
        "bvb": np.asarray(inputs["bv"], f32).astype(bf).reshape(1, DM),
        "bob": np.asarray(inputs["bo"], f32).astype(bf).reshape(1, DM),
        **consts,
    }
    x = np.asarray(inputs["x"], f32)
    pos = np.asarray(inputs["rotary_pos_enc"], f32)
    mask = np.asarray(inputs["padding_mask"], np.int32)
    # cos/sin broadcast tables per seq half: trig[:,0,:]=cos, [:,1,:]=sin
    trig = []
    for hf in range(2):
        sl = slice(hf * SC, (hf + 1) * SC)
        tt = np.zeros((P, 2, SC), bf)
        tt[:, 0, :] = np.tile(pos[sl, 0, 32:64].T.astype(bf), (4, 1))
        tt[:, 1, :] = np.tile(pos[sl, 0, 0:32].T.astype(bf), (4, 1))
        trig.append(np.ascontiguousarray(tt))
    in_maps = []
    for c in range(N_CORES):
        b_, hf = c // 2, c % 2
        sl = slice(hf * SC, (hf + 1) * SC)
        xc = np.ascontiguousarray(x[b_, sl])                     # [2048,1024]
        xTc = np.ascontiguousarray(
            xc.T.astype(pdt).reshape(KT, P, 2, 1024).transpose(1, 2, 0, 3)
        )
        xnc = np.ascontiguousarray(
            xc.astype(bf).reshape(NST, P, DM).transpose(1, 0, 2)
        )
        mnegc = np.ones((1, SC + P), np.float32)
        mnegc[0, 0:SC] = mask[b_, sl].astype(f32) * -1e4
        in_maps.append(
            {
                "xT": xTc,
                "xn": xnc,
                "trig": trig[hf],
                "mnegc": np.ascontiguousarray(mnegc.astype(bf)),
                **shared,
            }
        )
    return in_maps


def run(inputs, **kwargs):
    nc = _program()
    in_maps = make_in_maps(inputs)
    res = run_bass_kernel_spmd(
        nc, in_maps, core_ids=list(range(N_CORES)), **kwargs
    )
    out = np.zeros((B, S, DM), np.float32)
    for c in range(N_CORES):
        b_, hf = c // 2, c % 2
        out[b_, hf * SC:(hf + 1) * SC, :] = res.results[c]["y"]
    return out, res


def kernel(**inputs) -> np.ndarray:
    out, _ = run(inputs)
    return out
